# revision 8
# baseline (speedup 1.0000x reference)
"""Trainium2 Bass kernel for nn_MinDistTwoCircles.

kernel(**inputs) takes FULL unsharded inputs (c1,r1,z1,c2,r2,z2,num_iter)
and returns the FULL output tuple (bd, bi, t1, t2, p1, p2), matching
reference.reference().

Strategy:
  - Host (numpy): circle axes, initial angles (arctan2), per-pair dot
    constants; final p1/p2 reconstruction. O(N) once.
  - Device (8 NeuronCores, data-parallel over the pair axis): the
    2*num_iter Adam iterations. Per core 16384 pairs as [128 x 128] f32
    tiles; quantities packed into column blocks so most vector ops run at
    FD 256-512. Per iteration: comparison-based angle wrap -> ACT Sin
    (only ACT table ever loaded) -> bilinear geometry via block streams ->
    rsqrt(d2) and rsqrt(sigma*V + eps^2) via ACT bitcast magic seed + 2
    Newton steps -> Adam update. All state stays in SBUF.

Raw bass (no Tile): the toolchain here rejects instructions with >1 sem
wait, so the kernel uses explicit single-wait ping-pong semaphores:
per iteration DVE incs (d2, xt, tw) alternate with ACT incs (sins,
rd-seed, z-seed).
"""
import os
import sys
import types
import numpy as np
from contextlib import ExitStack

import concourse.bass as bass
import concourse.mybir as mybir
from concourse.bass_utils import run_bass_kernel_spmd

F = np.float32
P = 128
FD = 128
NCORES = 8
B1, B2, EPS = 0.9, 0.999, 1e-8
MAGIC = 0x5F3759DF
PI = float(np.pi)
TWO_PI = float(2 * np.pi)
F32 = mybir.dt.float32
I32 = mybir.dt.int32
AL = mybir.AluOpType
AF = mybir.ActivationFunctionType


# --------------------------------------------------------------------------
# host-side math
# --------------------------------------------------------------------------
def _normalize(v):
    return v / np.linalg.norm(v, axis=1, keepdims=True).astype(F)


def _axes(normal):
    n = _normalize(normal.astype(F))
    z = np.broadcast_to(np.array([0.0, 0.0, 1.0], F), n.shape).copy()
    boo = (np.sum(n * z, axis=1) < 0.01)[:, None]
    z = np.where(boo, np.array([0.0, 1.0, 0.0], F), z)
    x = _normalize(np.cross(n, z).astype(F))
    y = _normalize(np.cross(n, x).astype(F))
    return n, x, y


def _host_precompute(c1, r1, z1, c2, r2, z2):
    z1n, x1, y1 = _axes(z1)
    z2n, x2, y2 = _axes(z2)
    v = _normalize((c2 - c1).astype(F))
    v1 = _normalize(v - np.sum(v * z1n, axis=1, keepdims=True) * z1n)
    t1 = np.arctan2(np.sum(y1 * v1, axis=1), np.sum(x1 * v1, axis=1)).astype(F)
    v2 = _normalize(-v - np.sum(-v * z2n, axis=1, keepdims=True) * z2n)
    t2 = np.arctan2(np.sum(y2 * v2, axis=1), np.sum(x2 * v2, axis=1)).astype(F)

    u1 = r1[:, None] * x1
    w1 = r1[:, None] * y1
    u2 = r2[:, None] * x2
    w2 = r2[:, None] * y2
    e = (c1 - c2).astype(F)
    dot = lambda a, b: np.sum(a * b, axis=1).astype(F)
    cst = dict(
        A1=dot(e, u1), Bb1=dot(e, w1), A2=dot(e, u2), Bb2=dot(e, w2),
        U=dot(u1, u2), V=dot(u1, w2), W=dot(w1, u2), X=dot(w1, w2),
        K=(dot(e, e) + r1 * r1 + r2 * r2).astype(F),
    )
    return t1, t2, cst, (x1, y1, x2, y2)


# --------------------------------------------------------------------------
# optional NTFF profiling hook
# --------------------------------------------------------------------------
def _install_ntff_hook():
    try:
        if "antenv.axon_hooks" not in sys.modules:
            mod = types.ModuleType("antenv.axon_hooks")
            mod._hook = None
            mod.set_axon_ntff_profile_hook = lambda h: setattr(mod, "_hook", h)
            mod.get_axon_ntff_profile_hook = lambda: mod._hook
            import antenv
            antenv.axon_hooks = mod
            sys.modules["antenv.axon_hooks"] = mod
        from trn_agent_boot.trn_boot import _ntff_profile_via_ctypes
        sys.modules["antenv.axon_hooks"].set_axon_ntff_profile_hook(
            _ntff_profile_via_ctypes("/opt/axon/libaxon_pjrt.so"))
        return True
    except Exception:
        return False


# --------------------------------------------------------------------------
# device kernel
# --------------------------------------------------------------------------
def _blocks(tile_ap, offset_cols, dims):
    """AP keeping the partition dim, free dims replaced by [step,count]
    pairs (element units), starting at column offset_cols."""
    return bass.AP(
        tensor=tile_ap.tensor,
        offset=tile_ap.offset + offset_cols,
        ap=[list(tile_ap.ap[0])] + [list(d) for d in dims],
    )


def build_kernel(num_iter: int):
    # detect_race_conditions=False: the CoreSim race detector has no model of
    # same-engine in-order retirement (verified safe on HW), and this kernel
    # chains dependent ops on one engine constantly.
    nc = bass.Bass("TRN2", debug=False, detect_race_conditions=False)

    ca_d = nc.dram_tensor("ca", [P, 512], F32, kind="ExternalInput")
    cb_d = nc.dram_tensor("cb", [P, 512], F32, kind="ExternalInput")
    cd_d = nc.dram_tensor("cd", [P, 512], F32, kind="ExternalInput")
    ab_d = nc.dram_tensor("ab", [P, 256], F32, kind="ExternalInput")
    kt_d = nc.dram_tensor("kt", [P, 128], F32, kind="ExternalInput")
    tt_d = nc.dram_tensor("tt0", [P, 256], F32, kind="ExternalInput")
    mg_d = nc.dram_tensor("mgk", [P, 4], F32, kind="ExternalInput")

    t12_o = nc.dram_tensor("t12", [P, 256], F32, kind="ExternalOutput")
    sd_o = nc.dram_tensor("sd", [P, 256], F32, kind="ExternalOutput")
    tg_o = nc.dram_tensor("tg", [P, 512], F32, kind="ExternalOutput")

    ctx = ExitStack()
    sb = lambda name, cols, dt=F32: ctx.enter_context(
        nc.sbuf_tensor(name, [P, cols], dt))

    CA = sb("CA", 512); CB = sb("CB", 512); CD = sb("CD", 512)
    BIGX = sb("BIGX", 768)
    KT = sb("KT", 128)
    TT = sb("TTs", 256); TW = sb("TW", 256)
    MG = sb("MG", 4)
    MM = sb("MM", 256); VV = sb("VV", 256)
    SD = sb("SD", 256)
    G = sb("G", 256); GSQ = sb("GSQ", 256)
    TRIG4 = sb("TRIG4", 512)
    ABS = sb("ABS", 256)
    PPt = sb("PPt", 512); QQt = sb("QQt", 512)
    PROD = sb("PROD", 512); DSUM = sb("DSUM", 128)
    T1t = sb("T1t", 256); T2t = sb("T2t", 256); Nt = sb("Nt", 256)
    DM = sb("DM", 256)
    SEEDI = sb("SEEDI", 128, I32)
    ZSEEDI = sb("ZSEEDI", 256, I32)
    RD = sb("RD", 128); Q1 = sb("Q1", 128)
    ZZ = sb("ZZ", 256); QZ = sb("QZ", 256)
    XT = sb("XT", 256)
    UPD = sb("UPD", 256)
    V_ = sb("V_", 256); C1t = sb("C1t", 256); C2t = sb("C2t", 256)
    S_ = sb("S_", 256)

    dma = ctx.enter_context(nc.semaphore())
    s_d = ctx.enter_context(nc.semaphore())
    s_a = ctx.enter_context(nc.semaphore())
    s_g = ctx.enter_context(nc.semaphore())
    block = ctx.enter_context(nc.Block())

    iters = []
    lr = 0.1
    for phase in range(2):
        lr = lr / 10.0
        for i in range(num_iter):
            st = i + 1
            bc1 = 1 - B1 ** st
            bc2 = 1 - B2 ** st
            iters.append(dict(
                k=phase * num_iter + i, i=i,
                alpha=float(F(lr * (1 - B1) / bc1)),
                sigma=float(F((1 - B2) / bc2)),
            ))
    NIT = len(iters)
    BD2_INIT = float(F(99999.0) * F(99999.0))
    N_LOADS = 8

    @block.sync
    def _(sync):
        sync.dma_start(CA[:], ca_d.ap()[:]).then_inc(dma, 16)
        sync.dma_start(CB[:], cb_d.ap()[:]).then_inc(dma, 16)
        sync.dma_start(CD[:], cd_d.ap()[:]).then_inc(dma, 16)
        sync.dma_start(BIGX[:, 512:768], ab_d.ap()[:]).then_inc(dma, 16)
        sync.dma_start(KT[:], kt_d.ap()[:]).then_inc(dma, 16)
        sync.dma_start(TT[:], tt_d.ap()[:]).then_inc(dma, 16)
        sync.dma_start(TW[:], tt_d.ap()[:]).then_inc(dma, 16)
        sync.dma_start(MG[:], mg_d.ap()[:]).then_inc(dma, 16)
        sync.wait_ge(s_d, 3 * NIT)
        sync.dma_start(t12_o.ap()[:], TT[:]).then_inc(dma, 16)
        sync.dma_start(sd_o.ap()[:], SD[:]).then_inc(dma, 16)
        sync.dma_start(tg_o.ap()[:], TRIG4[:]).then_inc(dma, 16)

    @block.gpsimd
    def _(gpsimd):
        gpsimd.memset(MM[:], 0.0)
        gpsimd.memset(VV[:], 0.0)
        gpsimd.memset(SD[:, 128:256], 0.0)
        gpsimd.memset(SD[:, 0:128], BD2_INIT).then_inc(s_g, 1)

    @block.scalar
    def _(scalar):
        mg_magic = MG[:, 0:1]
        mg_mhalf = MG[:, 1:2]
        mg_pih = MG[:, 2:3]
        mg_m1 = MG[:, 3:4]
        for it in iters:
            k = it["k"]
            if k == 0:
                scalar.wait_ge(dma, 16 * N_LOADS)
            else:
                scalar.wait_ge(s_d, 3 * k)
            # sin half first (doesn't need ABS): TRIG4[256:512] = [si2|si1]
            twswap = _blocks(TW[:], 128, [[-128, 2], [1, 128]])
            nc.scalar.activation(TRIG4[:, 256:512], twswap,
                                 AF.Sin).then_inc(s_a, 1)
            # cos half: TRIG4[0:256] = Sin(pi/2 - [|tw2|,|tw1|]) = [co2|co1]
            nc.scalar.activation(ABS[:], TW[:], AF.Abs)
            absswap = _blocks(ABS[:], 128, [[-128, 2], [1, 128]])
            nc.scalar.activation(TRIG4[:, 0:256], absswap, AF.Sin,
                                 bias=mg_pih, scale=mg_m1).then_inc(s_a, 1)
            scalar.wait_ge(s_d, 3 * k + 1)
            nc.scalar.activation(SEEDI[:], DM[:, 0:128].bitcast(I32),
                                 AF.Identity, bias=mg_magic,
                                 scale=mg_mhalf).then_inc(s_a, 1)
            scalar.wait_ge(s_d, 3 * k + 2)
            nc.scalar.activation(ZSEEDI[:], XT[:].bitcast(I32),
                                 AF.Identity, bias=mg_magic,
                                 scale=mg_mhalf).then_inc(s_a, 1)

    @block.vector
    def _(vector):
        vector.wait_ge(s_g, 1)

        def tt(out, a, b, op):
            return nc.vector.tensor_tensor(out=out, in0=a, in1=b, op=op)

        def ts(out, a, s1, s2=None, op0=AL.mult, op1=None):
            if op1 is None:
                return nc.vector.tensor_scalar(out=out, in0=a, scalar1=s1,
                                               scalar2=None, op0=op0)
            return nc.vector.tensor_scalar(out=out, in0=a, scalar1=s1,
                                           scalar2=s2, op0=op0, op1=op1)

        def stt(out, a, s, b, op0, op1):
            return nc.vector.scalar_tensor_tensor(out=out, in0=a, scalar=s,
                                                  in1=b, op0=op0, op1=op1)

        def nr_step(ynew, y, x, q):
            tt(q, y, y, AL.mult)
            tt(q, q, x, AL.mult)
            ts(q, q, -0.5, 1.5, AL.mult, AL.add)
            return tt(ynew, y, q, AL.mult)

        for it in iters:
            k = it["k"]
            last = (k == NIT - 1)
            # ---- geometry ----
            # sin half ready first: QQ + (CD - QQ) overlap ACT's Abs+cos
            vector.wait_ge(s_a, 4 * k + 1)
            si4 = _blocks(TRIG4[:], 256, [[128, 2], [0, 2], [1, 128]])
            tt(QQt[:], si4, CB[:], AL.mult)
            tt(PPt[:], CD[:], QQt[:], AL.subtract)
            vector.wait_ge(s_a, 4 * k + 2)
            co4 = _blocks(TRIG4[:], 0, [[128, 2], [0, 2], [1, 128]])
            tt(QQt[:], co4, CA[:], AL.mult)
            tt(BIGX[:, 0:512], PPt[:], QQt[:], AL.subtract)
            co12 = _blocks(TRIG4[:], 128, [[-128, 2], [1, 128]])
            si12 = _blocks(TRIG4[:], 384, [[-128, 2], [1, 128]])
            b12 = _blocks(BIGX[:], 128, [[256, 2], [1, 128]])
            a12 = _blocks(BIGX[:], 0, [[256, 2], [1, 128]])
            tt(T1t[:], co12, b12, AL.mult)
            tt(T2t[:], si12, a12, AL.mult)
            tt(Nt[:], T1t[:], T2t[:], AL.subtract)
            # d2 = 2*(co1*A1 + si1*B1 + co2*a2' + si2*b2') + K via pair ops
            Aa = _blocks(BIGX[:], 512, [[-256, 2], [1, 128]])
            Bb = _blocks(BIGX[:], 640, [[-256, 2], [1, 128]])
            tt(T1t[:], co12, Aa, AL.mult)
            tt(T2t[:], si12, Bb, AL.mult)
            tt(T1t[:], T1t[:], T2t[:], AL.add)
            tt(DSUM[:], T1t[:, 0:128], T1t[:, 128:256], AL.add)
            stt(DM[:, 0:128], DSUM[:], 2.0, KT[:], AL.mult, AL.add)
            # clamp: this d2 form can round negative near circle contact
            # (the reference's sum-of-squares form cannot); below ~1e-8 it
            # has no relative accuracy anyway.
            ts(DM[:, 0:128], DM[:, 0:128], 1e-8, None,
               AL.max).then_inc(s_d, 1)                    # s_d -> 3k+1
            # fill while ACT computes the rd seed:
            tt(DM[:, 128:256], DM[:, 0:128], SD[:, 0:128], AL.is_lt)
            ts(DM[:, 128:256], DM[:, 128:256], -float(k))
            # ---- rd = rsqrt(d2); g ----
            vector.wait_ge(s_a, 4 * k + 3)
            seedf = SEEDI[:].bitcast(F32)
            nr_step(RD[:], seedf, DM[:, 0:128], Q1[:])
            nr_step(RD[:], RD[:], DM[:, 0:128], Q1[:])
            rdrep = _blocks(RD[:], 0, [[0, 2], [1, 128]])
            tt(G[:], Nt[:], rdrep, AL.mult)
            # ---- Adam state ----
            if it["i"] == 0:
                ts(MM[:], MM[:], 0.0)
                ts(VV[:], VV[:], 0.0)
            stt(MM[:], MM[:], float(B1), G[:], AL.mult, AL.add)
            tt(GSQ[:], G[:], G[:], AL.mult)
            stt(VV[:], VV[:], float(B2), GSQ[:], AL.mult, AL.add)
            ts(XT[:], VV[:], it["sigma"], 1e-16, AL.mult,
               AL.add).then_inc(s_d, 1)                    # s_d -> 3k+2
            # ---- z = rsqrt(xt); update ----
            vector.wait_ge(s_a, 4 * k + 4)
            zseedf = ZSEEDI[:].bitcast(F32)
            nr_step(ZZ[:], zseedf, XT[:], QZ[:])
            nr_step(ZZ[:], ZZ[:], XT[:], QZ[:])
            stt(UPD[:], MM[:], it["alpha"], ZZ[:], AL.mult, AL.mult)
            if last:
                tt(TT[:], TT[:], UPD[:], AL.subtract)
                tt(SD[:], SD[:], DM[:], AL.min).then_inc(s_d, 1)
            else:
                tt(V_[:], TW[:], UPD[:], AL.subtract)
                ts(C1t[:], V_[:], PI, -TWO_PI, AL.is_gt, AL.mult)
                ts(C2t[:], V_[:], -PI, TWO_PI, AL.is_lt, AL.mult)
                tt(S_[:], C1t[:], C2t[:], AL.add)
                tt(TW[:], V_[:], S_[:], AL.add).then_inc(s_d, 1)  # -> 3k+3
                # deferred, fills the gap while ACT runs Abs+Sin of k+1:
                tt(TT[:], TT[:], UPD[:], AL.subtract)
                tt(SD[:], SD[:], DM[:], AL.min)

    ctx.close()
    return nc


_BUILD_CACHE = {}


def _get_built(num_iter):
    if num_iter not in _BUILD_CACHE:
        _BUILD_CACHE[num_iter] = build_kernel(num_iter)
    return _BUILD_CACHE[num_iter]


def kernel(c1, r1, z1, c2, r2, z2, num_iter):
    num_iter = int(num_iter)
    c1 = np.asarray(c1, F); r1 = np.asarray(r1, F); z1 = np.asarray(z1, F)
    c2 = np.asarray(c2, F); r2 = np.asarray(r2, F); z2 = np.asarray(z2, F)
    N = c1.shape[0]
    per = N // NCORES
    assert per == P * FD, f"kernel hardcodes {P*FD} pairs/core, got {per}"

    t1, t2, cst, (x1, y1, x2, y2) = _host_precompute(c1, r1, z1, c2, r2, z2)

    def shard_pack(*qs):
        out = []
        for c in range(NCORES):
            sl = slice(c * per, (c + 1) * per)
            out.append(np.concatenate(
                [q[sl].reshape(P, FD) for q in qs], axis=1))
        return out

    A1, Bb1, A2, Bb2 = cst["A1"], cst["Bb1"], cst["A2"], cst["Bb2"]
    U, V, W, X, K = cst["U"], cst["V"], cst["W"], cst["X"], cst["K"]

    ca = shard_pack(U, W, U, V)
    cb = shard_pack(V, X, W, X)
    cd = shard_pack(A1, Bb1, -A2, -Bb2)
    ab = shard_pack(A1, Bb1)
    kt = shard_pack(K)
    tt0 = shard_pack(t1, t2)
    mgk = np.broadcast_to(
        np.array([[float(MAGIC), -0.5, PI / 2, -1.0]], F), (P, 4)).copy()

    in_maps = [
        {"ca": ca[c], "cb": cb[c], "cd": cd[c], "ab": ab[c], "kt": kt[c],
         "tt0": tt0[c], "mgk": mgk}
        for c in range(NCORES)
    ]

    nc = _get_built(num_iter)
    trace = os.environ.get("BASS_KERNEL_TRACE", "0") == "1"
    if trace:
        _install_ntff_hook()
    res = run_bass_kernel_spmd(nc, in_maps, core_ids=list(range(NCORES)),
                               trace=trace)
    if trace and res.exec_time_ns is not None:
        print(f"HW exec time: {res.exec_time_ns} ns")

    t1f = np.empty(N, F); t2f = np.empty(N, F)
    bd = np.empty(N, F); bi = np.empty(N, np.int32)
    co1 = np.empty(N, F); si1 = np.empty(N, F)
    co2 = np.empty(N, F); si2 = np.empty(N, F)
    for c in range(NCORES):
        sl = slice(c * per, (c + 1) * per)
        r = res.results[c]
        t12 = r["t12"]; sdv = r["sd"]; tg = r["tg"]
        t1f[sl] = t12[:, 0:128].reshape(-1)
        t2f[sl] = t12[:, 128:256].reshape(-1)
        bd[sl] = np.sqrt(sdv[:, 0:128].astype(np.float64)).astype(F).reshape(-1)
        bi[sl] = (-sdv[:, 128:256].reshape(-1)).astype(np.int32)
        co2[sl] = tg[:, 0:128].reshape(-1)
        co1[sl] = tg[:, 128:256].reshape(-1)
        si2[sl] = tg[:, 256:384].reshape(-1)
        si1[sl] = tg[:, 384:512].reshape(-1)

    p1 = c1 + r1[:, None] * (co1[:, None] * x1 + si1[:, None] * y1)
    p2 = c2 + r2[:, None] * (co2[:, None] * x2 + si2[:, None] * y2)
    return (bd, bi, t1f, t2f, p1.astype(F), p2.astype(F))


# revision 9
# speedup vs baseline: 1.1307x; 1.1307x over previous
"""Trainium2 Bass kernel for nn_MinDistTwoCircles.

kernel(**inputs) takes FULL unsharded inputs (c1,r1,z1,c2,r2,z2,num_iter)
and returns the FULL output tuple (bd, bi, t1, t2, p1, p2), matching
reference.reference().

Strategy:
  - Host (numpy): circle axes, initial angles (arctan2), per-pair dot
    constants; final p1/p2 reconstruction. O(N) once.
  - Device (8 NeuronCores, data-parallel over the pair axis): the
    2*num_iter Adam iterations. Per core 16384 pairs as [128 x 128] f32
    tiles; quantities packed into column blocks so most vector ops run at
    FD 256-512. Per iteration: comparison-based angle wrap -> ACT Sin
    (only ACT table ever loaded) -> bilinear geometry via block streams ->
    rsqrt(d2) and rsqrt(sigma*V + eps^2) via ACT bitcast magic seed + 2
    Newton steps -> Adam update. All state stays in SBUF.

Raw bass (no Tile): the toolchain here rejects instructions with >1 sem
wait, so the kernel uses explicit single-wait ping-pong semaphores:
per iteration DVE incs (d2, xt, tw) alternate with ACT incs (sins,
rd-seed, z-seed).
"""
import os
import sys
import types
import numpy as np
from contextlib import ExitStack

import concourse.bass as bass
import concourse.mybir as mybir
from concourse.bass_utils import run_bass_kernel_spmd

F = np.float32
P = 128
FD = 128
NCORES = 8
B1, B2, EPS = 0.9, 0.999, 1e-8
MAGIC = 0x5F3759DF
PI = float(np.pi)
TWO_PI = float(2 * np.pi)
F32 = mybir.dt.float32
I32 = mybir.dt.int32
AL = mybir.AluOpType
AF = mybir.ActivationFunctionType


# --------------------------------------------------------------------------
# host-side math
# --------------------------------------------------------------------------
def _normalize(v):
    return v / np.linalg.norm(v, axis=1, keepdims=True).astype(F)


def _axes(normal):
    n = _normalize(normal.astype(F))
    z = np.broadcast_to(np.array([0.0, 0.0, 1.0], F), n.shape).copy()
    boo = (np.sum(n * z, axis=1) < 0.01)[:, None]
    z = np.where(boo, np.array([0.0, 1.0, 0.0], F), z)
    x = _normalize(np.cross(n, z).astype(F))
    y = _normalize(np.cross(n, x).astype(F))
    return n, x, y


def _host_precompute(c1, r1, z1, c2, r2, z2):
    z1n, x1, y1 = _axes(z1)
    z2n, x2, y2 = _axes(z2)
    v = _normalize((c2 - c1).astype(F))
    v1 = _normalize(v - np.sum(v * z1n, axis=1, keepdims=True) * z1n)
    t1 = np.arctan2(np.sum(y1 * v1, axis=1), np.sum(x1 * v1, axis=1)).astype(F)
    v2 = _normalize(-v - np.sum(-v * z2n, axis=1, keepdims=True) * z2n)
    t2 = np.arctan2(np.sum(y2 * v2, axis=1), np.sum(x2 * v2, axis=1)).astype(F)

    u1 = r1[:, None] * x1
    w1 = r1[:, None] * y1
    u2 = r2[:, None] * x2
    w2 = r2[:, None] * y2
    e = (c1 - c2).astype(F)
    dot = lambda a, b: np.sum(a * b, axis=1).astype(F)
    cst = dict(
        A1=dot(e, u1), Bb1=dot(e, w1), A2=dot(e, u2), Bb2=dot(e, w2),
        U=dot(u1, u2), V=dot(u1, w2), W=dot(w1, u2), X=dot(w1, w2),
        K=(dot(e, e) + r1 * r1 + r2 * r2).astype(F),
    )
    return t1, t2, cst, (x1, y1, x2, y2)


# --------------------------------------------------------------------------
# optional NTFF profiling hook
# --------------------------------------------------------------------------
def _install_ntff_hook():
    try:
        if "antenv.axon_hooks" not in sys.modules:
            mod = types.ModuleType("antenv.axon_hooks")
            mod._hook = None
            mod.set_axon_ntff_profile_hook = lambda h: setattr(mod, "_hook", h)
            mod.get_axon_ntff_profile_hook = lambda: mod._hook
            import antenv
            antenv.axon_hooks = mod
            sys.modules["antenv.axon_hooks"] = mod
        from trn_agent_boot.trn_boot import _ntff_profile_via_ctypes
        sys.modules["antenv.axon_hooks"].set_axon_ntff_profile_hook(
            _ntff_profile_via_ctypes("/opt/axon/libaxon_pjrt.so"))
        return True
    except Exception:
        return False


# --------------------------------------------------------------------------
# device kernel
# --------------------------------------------------------------------------
def _blocks(tile_ap, offset_cols, dims):
    """AP keeping the partition dim, free dims replaced by [step,count]
    pairs (element units), starting at column offset_cols."""
    return bass.AP(
        tensor=tile_ap.tensor,
        offset=tile_ap.offset + offset_cols,
        ap=[list(tile_ap.ap[0])] + [list(d) for d in dims],
    )


def build_kernel(num_iter: int):
    # detect_race_conditions=False: the CoreSim race detector has no model of
    # same-engine in-order retirement (verified safe on HW), and this kernel
    # chains dependent ops on one engine constantly.
    nc = bass.Bass("TRN2", debug=False, detect_race_conditions=False)

    ca_d = nc.dram_tensor("ca", [P, 512], F32, kind="ExternalInput")
    cb_d = nc.dram_tensor("cb", [P, 512], F32, kind="ExternalInput")
    cd_d = nc.dram_tensor("cd", [P, 512], F32, kind="ExternalInput")
    ab_d = nc.dram_tensor("ab", [P, 256], F32, kind="ExternalInput")
    kt_d = nc.dram_tensor("kt", [P, 128], F32, kind="ExternalInput")
    tt_d = nc.dram_tensor("tt0", [P, 256], F32, kind="ExternalInput")
    mg_d = nc.dram_tensor("mgk", [P, 4], F32, kind="ExternalInput")

    t12_o = nc.dram_tensor("t12", [P, 256], F32, kind="ExternalOutput")
    sd_o = nc.dram_tensor("sd", [P, 256], F32, kind="ExternalOutput")
    tg_o = nc.dram_tensor("tg", [P, 512], F32, kind="ExternalOutput")

    ctx = ExitStack()
    sb = lambda name, cols, dt=F32: ctx.enter_context(
        nc.sbuf_tensor(name, [P, cols], dt))

    CA = sb("CA", 512); CB = sb("CB", 512); CD = sb("CD", 512)
    BIGX = sb("BIGX", 768)
    KT = sb("KT", 128)
    TT = sb("TTs", 256); TW = sb("TW", 256)
    MG = sb("MG", 4)
    MM = sb("MM", 256); VV = sb("VV", 256)
    SD = sb("SD", 256)
    G = sb("G", 256); GSQ = sb("GSQ", 256)
    TRIG4 = sb("TRIG4", 512)
    ABS = sb("ABS", 256)
    PPt = sb("PPt", 512); QQt = sb("QQt", 512)
    PROD = sb("PROD", 512); DSUM = sb("DSUM", 128)
    T1t = sb("T1t", 256); T2t = sb("T2t", 256); Nt = sb("Nt", 256)
    DM = sb("DM", 256)
    SEEDI = sb("SEEDI", 128, I32)
    ZSEEDI = sb("ZSEEDI", 256, I32)
    RD = sb("RD", 128); Q1 = sb("Q1", 128)
    ZZ = sb("ZZ", 256); QZ = sb("QZ", 256)
    XT = sb("XT", 256)
    UPD = sb("UPD", 256)
    V_ = sb("V_", 256); C1t = sb("C1t", 256); C2t = sb("C2t", 256)
    S_ = sb("S_", 256)

    dma = ctx.enter_context(nc.semaphore())
    s_d = ctx.enter_context(nc.semaphore())
    s_a = ctx.enter_context(nc.semaphore())
    s_g = ctx.enter_context(nc.semaphore())
    block = ctx.enter_context(nc.Block())

    iters = []
    lr = 0.1
    for phase in range(2):
        lr = lr / 10.0
        for i in range(num_iter):
            st = i + 1
            bc1 = 1 - B1 ** st
            bc2 = 1 - B2 ** st
            iters.append(dict(
                k=phase * num_iter + i, i=i,
                alpha=float(F(lr * (1 - B1) / bc1)),
                sigma=float(F((1 - B2) / bc2)),
            ))
    NIT = len(iters)
    BD2_INIT = float(F(99999.0) * F(99999.0))
    N_LOADS = 8

    @block.sync
    def _(sync):
        sync.dma_start(CA[:], ca_d.ap()[:]).then_inc(dma, 16)
        sync.dma_start(CB[:], cb_d.ap()[:]).then_inc(dma, 16)
        sync.dma_start(CD[:], cd_d.ap()[:]).then_inc(dma, 16)
        sync.dma_start(BIGX[:, 512:768], ab_d.ap()[:]).then_inc(dma, 16)
        sync.dma_start(KT[:], kt_d.ap()[:]).then_inc(dma, 16)
        sync.dma_start(TT[:], tt_d.ap()[:]).then_inc(dma, 16)
        sync.dma_start(TW[:], tt_d.ap()[:]).then_inc(dma, 16)
        sync.dma_start(MG[:], mg_d.ap()[:]).then_inc(dma, 16)
        sync.wait_ge(s_d, 3 * NIT)
        sync.dma_start(t12_o.ap()[:], TT[:]).then_inc(dma, 16)
        sync.dma_start(sd_o.ap()[:], SD[:]).then_inc(dma, 16)
        sync.dma_start(tg_o.ap()[:], TRIG4[:]).then_inc(dma, 16)

    @block.gpsimd
    def _(gpsimd):
        gpsimd.memset(MM[:], 0.0)
        gpsimd.memset(VV[:], 0.0)
        gpsimd.memset(SD[:, 128:256], 0.0)
        gpsimd.memset(SD[:, 0:128], BD2_INIT).then_inc(s_g, 1)

    @block.scalar
    def _(scalar):
        mg_magic = MG[:, 0:1]
        mg_mhalf = MG[:, 1:2]
        mg_pih = MG[:, 2:3]
        mg_m1 = MG[:, 3:4]
        for it in iters:
            k = it["k"]
            if k == 0:
                scalar.wait_ge(dma, 16 * N_LOADS)
            else:
                scalar.wait_ge(s_d, 3 * k)
            # sin half: TRIG4[256:512] = [si2|si1]
            twswap = _blocks(TW[:], 128, [[-128, 2], [1, 128]])
            nc.scalar.activation(TRIG4[:, 256:512], twswap, AF.Sin)
            # cos half: TRIG4[0:256] = Sin(pi/2 - [|tw2|,|tw1|]) = [co2|co1]
            nc.scalar.activation(ABS[:], TW[:], AF.Abs)
            absswap = _blocks(ABS[:], 128, [[-128, 2], [1, 128]])
            nc.scalar.activation(TRIG4[:, 0:256], absswap, AF.Sin,
                                 bias=mg_pih, scale=mg_m1).then_inc(s_a, 2)
            scalar.wait_ge(s_d, 3 * k + 1)
            nc.scalar.activation(SEEDI[:], DM[:, 0:128].bitcast(I32),
                                 AF.Identity, bias=mg_magic,
                                 scale=mg_mhalf).then_inc(s_a, 1)
            scalar.wait_ge(s_d, 3 * k + 2)
            nc.scalar.activation(ZSEEDI[:], XT[:].bitcast(I32),
                                 AF.Identity, bias=mg_magic,
                                 scale=mg_mhalf).then_inc(s_a, 1)

    @block.vector
    def _(vector):
        vector.wait_ge(s_g, 1)

        def tt(out, a, b, op):
            return nc.vector.tensor_tensor(out=out, in0=a, in1=b, op=op)

        def ts(out, a, s1, s2=None, op0=AL.mult, op1=None):
            if op1 is None:
                return nc.vector.tensor_scalar(out=out, in0=a, scalar1=s1,
                                               scalar2=None, op0=op0)
            return nc.vector.tensor_scalar(out=out, in0=a, scalar1=s1,
                                           scalar2=s2, op0=op0, op1=op1)

        def stt(out, a, s, b, op0, op1):
            return nc.vector.scalar_tensor_tensor(out=out, in0=a, scalar=s,
                                                  in1=b, op0=op0, op1=op1)

        def nr_step(ynew, y, x, q):
            tt(q, y, y, AL.mult)
            tt(q, q, x, AL.mult)
            ts(q, q, -0.5, 1.5, AL.mult, AL.add)
            return tt(ynew, y, q, AL.mult)

        for it in iters:
            k = it["k"]
            last = (k == NIT - 1)
            # ---- geometry ----
            vector.wait_ge(s_a, 4 * k + 2)
            si4 = _blocks(TRIG4[:], 256, [[128, 2], [0, 2], [1, 128]])
            tt(QQt[:], si4, CB[:], AL.mult)
            tt(PPt[:], CD[:], QQt[:], AL.subtract)
            co4 = _blocks(TRIG4[:], 0, [[128, 2], [0, 2], [1, 128]])
            tt(QQt[:], co4, CA[:], AL.mult)
            tt(BIGX[:, 0:512], PPt[:], QQt[:], AL.subtract)
            co12 = _blocks(TRIG4[:], 128, [[-128, 2], [1, 128]])
            si12 = _blocks(TRIG4[:], 384, [[-128, 2], [1, 128]])
            b12 = _blocks(BIGX[:], 128, [[256, 2], [1, 128]])
            a12 = _blocks(BIGX[:], 0, [[256, 2], [1, 128]])
            tt(T1t[:], co12, b12, AL.mult)
            tt(T2t[:], si12, a12, AL.mult)
            tt(Nt[:], T1t[:], T2t[:], AL.subtract)
            # d2 = 2*(co1*A1 + si1*B1 + co2*a2' + si2*b2') + K via pair ops
            Aa = _blocks(BIGX[:], 512, [[-256, 2], [1, 128]])
            Bb = _blocks(BIGX[:], 640, [[-256, 2], [1, 128]])
            tt(T1t[:], co12, Aa, AL.mult)
            tt(T2t[:], si12, Bb, AL.mult)
            tt(T1t[:], T1t[:], T2t[:], AL.add)
            tt(DSUM[:], T1t[:, 0:128], T1t[:, 128:256], AL.add)
            stt(DM[:, 0:128], DSUM[:], 2.0, KT[:], AL.mult, AL.add)
            # clamp: this d2 form can round negative near circle contact
            # (the reference's sum-of-squares form cannot); below ~1e-8 it
            # has no relative accuracy anyway.
            ts(DM[:, 0:128], DM[:, 0:128], 1e-8, None,
               AL.max).then_inc(s_d, 1)                    # s_d -> 3k+1
            # fill while ACT computes the rd seed:
            tt(DM[:, 128:256], DM[:, 0:128], SD[:, 0:128], AL.is_lt)
            ts(DM[:, 128:256], DM[:, 128:256], -float(k))
            # ---- rd = rsqrt(d2); g ----
            vector.wait_ge(s_a, 4 * k + 3)
            seedf = SEEDI[:].bitcast(F32)
            nr_step(RD[:], seedf, DM[:, 0:128], Q1[:])
            nr_step(RD[:], RD[:], DM[:, 0:128], Q1[:])
            rdrep = _blocks(RD[:], 0, [[0, 2], [1, 128]])
            tt(G[:], Nt[:], rdrep, AL.mult)
            # ---- Adam state ----
            if it["i"] == 0:
                ts(MM[:], MM[:], 0.0)
                ts(VV[:], VV[:], 0.0)
            stt(MM[:], MM[:], float(B1), G[:], AL.mult, AL.add)
            tt(GSQ[:], G[:], G[:], AL.mult)
            stt(VV[:], VV[:], float(B2), GSQ[:], AL.mult, AL.add)
            ts(XT[:], VV[:], it["sigma"], 1e-16, AL.mult,
               AL.add).then_inc(s_d, 1)                    # s_d -> 3k+2
            # ---- z = rsqrt(xt); update ----
            vector.wait_ge(s_a, 4 * k + 4)
            zseedf = ZSEEDI[:].bitcast(F32)
            nr_step(ZZ[:], zseedf, XT[:], QZ[:])
            nr_step(ZZ[:], ZZ[:], XT[:], QZ[:])
            stt(UPD[:], MM[:], it["alpha"], ZZ[:], AL.mult, AL.mult)
            if last:
                tt(TT[:], TT[:], UPD[:], AL.subtract)
                tt(SD[:], SD[:], DM[:], AL.min).then_inc(s_d, 1)
            else:
                tt(V_[:], TW[:], UPD[:], AL.subtract)
                ts(C1t[:], V_[:], PI, -TWO_PI, AL.is_gt, AL.mult)
                ts(C2t[:], V_[:], -PI, TWO_PI, AL.is_lt, AL.mult)
                tt(S_[:], C1t[:], C2t[:], AL.add)
                tt(TW[:], V_[:], S_[:], AL.add).then_inc(s_d, 1)  # -> 3k+3
                # deferred, fills the gap while ACT runs Abs+Sin of k+1:
                tt(TT[:], TT[:], UPD[:], AL.subtract)
                tt(SD[:], SD[:], DM[:], AL.min)

    ctx.close()
    return nc


_BUILD_CACHE = {}


def _get_built(num_iter):
    if num_iter not in _BUILD_CACHE:
        _BUILD_CACHE[num_iter] = build_kernel(num_iter)
    return _BUILD_CACHE[num_iter]


def kernel(c1, r1, z1, c2, r2, z2, num_iter):
    num_iter = int(num_iter)
    c1 = np.asarray(c1, F); r1 = np.asarray(r1, F); z1 = np.asarray(z1, F)
    c2 = np.asarray(c2, F); r2 = np.asarray(r2, F); z2 = np.asarray(z2, F)
    N = c1.shape[0]
    per = N // NCORES
    assert per == P * FD, f"kernel hardcodes {P*FD} pairs/core, got {per}"

    t1, t2, cst, (x1, y1, x2, y2) = _host_precompute(c1, r1, z1, c2, r2, z2)

    def shard_pack(*qs):
        out = []
        for c in range(NCORES):
            sl = slice(c * per, (c + 1) * per)
            out.append(np.concatenate(
                [q[sl].reshape(P, FD) for q in qs], axis=1))
        return out

    A1, Bb1, A2, Bb2 = cst["A1"], cst["Bb1"], cst["A2"], cst["Bb2"]
    U, V, W, X, K = cst["U"], cst["V"], cst["W"], cst["X"], cst["K"]

    ca = shard_pack(U, W, U, V)
    cb = shard_pack(V, X, W, X)
    cd = shard_pack(A1, Bb1, -A2, -Bb2)
    ab = shard_pack(A1, Bb1)
    kt = shard_pack(K)
    tt0 = shard_pack(t1, t2)
    mgk = np.broadcast_to(
        np.array([[float(MAGIC), -0.5, PI / 2, -1.0]], F), (P, 4)).copy()

    in_maps = [
        {"ca": ca[c], "cb": cb[c], "cd": cd[c], "ab": ab[c], "kt": kt[c],
         "tt0": tt0[c], "mgk": mgk}
        for c in range(NCORES)
    ]

    nc = _get_built(num_iter)
    trace = os.environ.get("BASS_KERNEL_TRACE", "0") == "1"
    if trace:
        _install_ntff_hook()
    res = run_bass_kernel_spmd(nc, in_maps, core_ids=list(range(NCORES)),
                               trace=trace)
    if trace and res.exec_time_ns is not None:
        print(f"HW exec time: {res.exec_time_ns} ns")

    t1f = np.empty(N, F); t2f = np.empty(N, F)
    bd = np.empty(N, F); bi = np.empty(N, np.int32)
    co1 = np.empty(N, F); si1 = np.empty(N, F)
    co2 = np.empty(N, F); si2 = np.empty(N, F)
    for c in range(NCORES):
        sl = slice(c * per, (c + 1) * per)
        r = res.results[c]
        t12 = r["t12"]; sdv = r["sd"]; tg = r["tg"]
        t1f[sl] = t12[:, 0:128].reshape(-1)
        t2f[sl] = t12[:, 128:256].reshape(-1)
        bd[sl] = np.sqrt(sdv[:, 0:128].astype(np.float64)).astype(F).reshape(-1)
        bi[sl] = (-sdv[:, 128:256].reshape(-1)).astype(np.int32)
        co2[sl] = tg[:, 0:128].reshape(-1)
        co1[sl] = tg[:, 128:256].reshape(-1)
        si2[sl] = tg[:, 256:384].reshape(-1)
        si1[sl] = tg[:, 384:512].reshape(-1)

    p1 = c1 + r1[:, None] * (co1[:, None] * x1 + si1[:, None] * y1)
    p2 = c2 + r2[:, None] * (co2[:, None] * x2 + si2[:, None] * y2)
    return (bd, bi, t1f, t2f, p1.astype(F), p2.astype(F))


# revision 10
# speedup vs baseline: 1.2140x; 1.0737x over previous
"""Trainium2 Bass kernel for nn_MinDistTwoCircles.

kernel(**inputs) takes FULL unsharded inputs (c1,r1,z1,c2,r2,z2,num_iter)
and returns the FULL output tuple (bd, bi, t1, t2, p1, p2), matching
reference.reference().

Strategy:
  - Host (numpy): circle axes, initial angles (arctan2), per-pair dot
    constants; final p1/p2 reconstruction. O(N) once.
  - Device (8 NeuronCores, data-parallel over the pair axis): the
    2*num_iter Adam iterations. Per core 16384 pairs as [128 x 128] f32
    tiles; quantities packed into column blocks so most vector ops run at
    FD 256-512. Per iteration: comparison-based angle wrap -> ACT Sin
    (only ACT table ever loaded) -> bilinear geometry via block streams ->
    rsqrt(d2) and rsqrt(sigma*V + eps^2) via ACT bitcast magic seed + 2
    Newton steps -> Adam update. All state stays in SBUF.

Raw bass (no Tile): the toolchain here rejects instructions with >1 sem
wait, so the kernel uses explicit single-wait ping-pong semaphores:
per iteration DVE incs (d2, xt, tw) alternate with ACT incs (sins,
rd-seed, z-seed).
"""
import os
import sys
import types
import numpy as np
from contextlib import ExitStack

import concourse.bass as bass
import concourse.mybir as mybir
from concourse.bass_utils import run_bass_kernel_spmd

F = np.float32
P = 128
FD = 128
NCORES = 8
B1, B2, EPS = 0.9, 0.999, 1e-8
MAGIC = 0x5F3759DF
PI = float(np.pi)
TWO_PI = float(2 * np.pi)
F32 = mybir.dt.float32
I32 = mybir.dt.int32
AL = mybir.AluOpType
AF = mybir.ActivationFunctionType


# --------------------------------------------------------------------------
# host-side math
# --------------------------------------------------------------------------
def _normalize(v):
    return v / np.linalg.norm(v, axis=1, keepdims=True).astype(F)


def _axes(normal):
    n = _normalize(normal.astype(F))
    z = np.broadcast_to(np.array([0.0, 0.0, 1.0], F), n.shape).copy()
    boo = (np.sum(n * z, axis=1) < 0.01)[:, None]
    z = np.where(boo, np.array([0.0, 1.0, 0.0], F), z)
    x = _normalize(np.cross(n, z).astype(F))
    y = _normalize(np.cross(n, x).astype(F))
    return n, x, y


def _host_precompute(c1, r1, z1, c2, r2, z2):
    z1n, x1, y1 = _axes(z1)
    z2n, x2, y2 = _axes(z2)
    v = _normalize((c2 - c1).astype(F))
    v1 = _normalize(v - np.sum(v * z1n, axis=1, keepdims=True) * z1n)
    t1 = np.arctan2(np.sum(y1 * v1, axis=1), np.sum(x1 * v1, axis=1)).astype(F)
    v2 = _normalize(-v - np.sum(-v * z2n, axis=1, keepdims=True) * z2n)
    t2 = np.arctan2(np.sum(y2 * v2, axis=1), np.sum(x2 * v2, axis=1)).astype(F)

    u1 = r1[:, None] * x1
    w1 = r1[:, None] * y1
    u2 = r2[:, None] * x2
    w2 = r2[:, None] * y2
    e = (c1 - c2).astype(F)
    dot = lambda a, b: np.sum(a * b, axis=1).astype(F)
    cst = dict(
        A1=dot(e, u1), Bb1=dot(e, w1), A2=dot(e, u2), Bb2=dot(e, w2),
        U=dot(u1, u2), V=dot(u1, w2), W=dot(w1, u2), X=dot(w1, w2),
        K=(dot(e, e) + r1 * r1 + r2 * r2).astype(F),
    )
    return t1, t2, cst, (x1, y1, x2, y2)


# --------------------------------------------------------------------------
# optional NTFF profiling hook
# --------------------------------------------------------------------------
def _install_ntff_hook():
    try:
        if "antenv.axon_hooks" not in sys.modules:
            mod = types.ModuleType("antenv.axon_hooks")
            mod._hook = None
            mod.set_axon_ntff_profile_hook = lambda h: setattr(mod, "_hook", h)
            mod.get_axon_ntff_profile_hook = lambda: mod._hook
            import antenv
            antenv.axon_hooks = mod
            sys.modules["antenv.axon_hooks"] = mod
        from trn_agent_boot.trn_boot import _ntff_profile_via_ctypes
        sys.modules["antenv.axon_hooks"].set_axon_ntff_profile_hook(
            _ntff_profile_via_ctypes("/opt/axon/libaxon_pjrt.so"))
        return True
    except Exception:
        return False


# --------------------------------------------------------------------------
# device kernel
# --------------------------------------------------------------------------
def _blocks(tile_ap, offset_cols, dims):
    """AP keeping the partition dim, free dims replaced by [step,count]
    pairs (element units), starting at column offset_cols."""
    return bass.AP(
        tensor=tile_ap.tensor,
        offset=tile_ap.offset + offset_cols,
        ap=[list(tile_ap.ap[0])] + [list(d) for d in dims],
    )


def build_kernel(num_iter: int):
    # detect_race_conditions=False: the CoreSim race detector has no model of
    # same-engine in-order retirement (verified safe on HW), and this kernel
    # chains dependent ops on one engine constantly.
    nc = bass.Bass("TRN2", debug=False, detect_race_conditions=False)

    ca_d = nc.dram_tensor("ca", [P, 512], F32, kind="ExternalInput")
    cb_d = nc.dram_tensor("cb", [P, 512], F32, kind="ExternalInput")
    cd_d = nc.dram_tensor("cd", [P, 512], F32, kind="ExternalInput")
    ab_d = nc.dram_tensor("ab", [P, 256], F32, kind="ExternalInput")
    kt_d = nc.dram_tensor("kt", [P, 128], F32, kind="ExternalInput")
    tt_d = nc.dram_tensor("tt0", [P, 256], F32, kind="ExternalInput")
    mg_d = nc.dram_tensor("mgk", [P, 4], F32, kind="ExternalInput")

    t12_o = nc.dram_tensor("t12", [P, 256], F32, kind="ExternalOutput")
    sd_o = nc.dram_tensor("sd", [P, 256], F32, kind="ExternalOutput")
    tg_o = nc.dram_tensor("tg", [P, 512], F32, kind="ExternalOutput")

    ctx = ExitStack()
    sb = lambda name, cols, dt=F32: ctx.enter_context(
        nc.sbuf_tensor(name, [P, cols], dt))

    CA = sb("CA", 512); CB = sb("CB", 512); CD = sb("CD", 512)
    BIGX = sb("BIGX", 768)
    KT = sb("KT", 128)
    TT = sb("TTs", 256); ASIN = sb("ASIN", 256)
    MG = sb("MG", 4)
    MM = sb("MM", 256); VV = sb("VV", 256)
    SD = sb("SD", 256)
    G = sb("G", 256); GSQ = sb("GSQ", 256)
    TRIG4 = sb("TRIG4", 512)
    ABS = sb("ABS", 256)
    PPt = sb("PPt", 512); QQt = sb("QQt", 512)
    PROD = sb("PROD", 512); DSUM = sb("DSUM", 128)
    T1t = sb("T1t", 256); T2t = sb("T2t", 256); Nt = sb("Nt", 256)
    DM = sb("DM", 256)
    SEEDI = sb("SEEDI", 128, I32)
    ZSEEDI = sb("ZSEEDI", 256, I32)
    RD = sb("RD", 128); Q1 = sb("Q1", 128); P1q = sb("P1q", 128)
    ZZ = sb("ZZ", 256); QZ = sb("QZ", 256); PZq = sb("PZq", 256)
    XT = sb("XT", 256)
    UPD = sb("UPD", 256)
    C1t = sb("C1t", 256); C2t = sb("C2t", 256)
    S_ = sb("S_", 256)

    dma = ctx.enter_context(nc.semaphore())
    s_d = ctx.enter_context(nc.semaphore())
    s_a = ctx.enter_context(nc.semaphore())
    s_g = ctx.enter_context(nc.semaphore())
    block = ctx.enter_context(nc.Block())

    iters = []
    lr = 0.1
    for phase in range(2):
        lr = lr / 10.0
        for i in range(num_iter):
            st = i + 1
            bc1 = 1 - B1 ** st
            bc2 = 1 - B2 ** st
            iters.append(dict(
                k=phase * num_iter + i, i=i,
                alpha=float(F(lr * (1 - B1) / bc1)),
                sigma=float(F((1 - B2) / bc2)),
            ))
    NIT = len(iters)
    BD2_INIT = float(F(99999.0) * F(99999.0))
    N_LOADS = 8

    @block.sync
    def _(sync):
        sync.dma_start(CA[:], ca_d.ap()[:]).then_inc(dma, 16)
        sync.dma_start(CB[:], cb_d.ap()[:]).then_inc(dma, 16)
        sync.dma_start(CD[:], cd_d.ap()[:]).then_inc(dma, 16)
        sync.dma_start(BIGX[:, 512:768], ab_d.ap()[:]).then_inc(dma, 16)
        sync.dma_start(KT[:], kt_d.ap()[:]).then_inc(dma, 16)
        sync.dma_start(TT[:], tt_d.ap()[:]).then_inc(dma, 16)
        sync.dma_start(ASIN[:], tt_d.ap()[:]).then_inc(dma, 16)
        sync.dma_start(MG[:], mg_d.ap()[:]).then_inc(dma, 16)
        sync.wait_ge(s_d, 3 * NIT)
        sync.dma_start(t12_o.ap()[:], TT[:]).then_inc(dma, 16)
        sync.dma_start(sd_o.ap()[:], SD[:]).then_inc(dma, 16)
        sync.dma_start(tg_o.ap()[:], TRIG4[:]).then_inc(dma, 16)

    @block.gpsimd
    def _(gpsimd):
        gpsimd.memset(MM[:], 0.0)
        gpsimd.memset(VV[:], 0.0)
        gpsimd.memset(SD[:, 128:256], 0.0)
        gpsimd.memset(SD[:, 0:128], BD2_INIT).then_inc(s_g, 1)

    @block.scalar
    def _(scalar):
        mg_magic = MG[:, 0:1]
        mg_mhalf = MG[:, 1:2]
        mg_pih = MG[:, 2:3]
        mg_m1 = MG[:, 3:4]
        for it in iters:
            k = it["k"]
            if k == 0:
                scalar.wait_ge(dma, 16 * N_LOADS)
            else:
                scalar.wait_ge(s_d, 3 * k)
            # sin half: TRIG4[256:512] = [si2|si1]
            twswap = _blocks(ASIN[:], 128, [[-128, 2], [1, 128]])
            nc.scalar.activation(TRIG4[:, 256:512], twswap, AF.Sin)
            # cos half: TRIG4[0:256] = Sin(pi/2 - [|tw2|,|tw1|]) = [co2|co1]
            nc.scalar.activation(ABS[:], ASIN[:], AF.Abs)
            absswap = _blocks(ABS[:], 128, [[-128, 2], [1, 128]])
            nc.scalar.activation(TRIG4[:, 0:256], absswap, AF.Sin,
                                 bias=mg_pih, scale=mg_m1).then_inc(s_a, 2)
            scalar.wait_ge(s_d, 3 * k + 1)
            nc.scalar.activation(SEEDI[:], DM[:, 0:128].bitcast(I32),
                                 AF.Identity, bias=mg_magic,
                                 scale=mg_mhalf).then_inc(s_a, 1)
            scalar.wait_ge(s_d, 3 * k + 2)
            nc.scalar.activation(ZSEEDI[:], XT[:].bitcast(I32),
                                 AF.Identity, bias=mg_magic,
                                 scale=mg_mhalf).then_inc(s_a, 1)

    @block.vector
    def _(vector):
        vector.wait_ge(s_g, 1)

        def tt(out, a, b, op):
            return nc.vector.tensor_tensor(out=out, in0=a, in1=b, op=op)

        def ts(out, a, s1, s2=None, op0=AL.mult, op1=None):
            if op1 is None:
                return nc.vector.tensor_scalar(out=out, in0=a, scalar1=s1,
                                               scalar2=None, op0=op0)
            return nc.vector.tensor_scalar(out=out, in0=a, scalar1=s1,
                                           scalar2=s2, op0=op0, op1=op1)

        def stt(out, a, s, b, op0, op1):
            return nc.vector.scalar_tensor_tensor(out=out, in0=a, scalar=s,
                                                  in1=b, op0=op0, op1=op1)

        def cubic_rsqrt(ynew, y, x, q, p):
            # one 2nd-order Householder step: y*(15/8 - 5/4 q + 3/8 q^2)
            tt(q, y, y, AL.mult)
            tt(q, q, x, AL.mult)
            ts(p, q, 0.375, -1.25, AL.mult, AL.add)
            tt(p, p, q, AL.mult)
            ts(p, p, 1.875, None, AL.add)
            return tt(ynew, y, p, AL.mult)

        for it in iters:
            k = it["k"]
            last = (k == NIT - 1)
            # ---- geometry ----
            vector.wait_ge(s_a, 4 * k + 2)
            si4 = _blocks(TRIG4[:], 256, [[128, 2], [0, 2], [1, 128]])
            tt(QQt[:], si4, CB[:], AL.mult)
            tt(PPt[:], CD[:], QQt[:], AL.subtract)
            co4 = _blocks(TRIG4[:], 0, [[128, 2], [0, 2], [1, 128]])
            tt(QQt[:], co4, CA[:], AL.mult)
            tt(BIGX[:, 0:512], PPt[:], QQt[:], AL.subtract)
            co12 = _blocks(TRIG4[:], 128, [[-128, 2], [1, 128]])
            si12 = _blocks(TRIG4[:], 384, [[-128, 2], [1, 128]])
            b12 = _blocks(BIGX[:], 128, [[256, 2], [1, 128]])
            a12 = _blocks(BIGX[:], 0, [[256, 2], [1, 128]])
            tt(T1t[:], co12, b12, AL.mult)
            tt(T2t[:], si12, a12, AL.mult)
            tt(Nt[:], T1t[:], T2t[:], AL.subtract)
            # d2 = 2*(co1*A1 + si1*B1 + co2*a2' + si2*b2') + K via pair ops
            Aa = _blocks(BIGX[:], 512, [[-256, 2], [1, 128]])
            Bb = _blocks(BIGX[:], 640, [[-256, 2], [1, 128]])
            tt(T1t[:], co12, Aa, AL.mult)
            tt(T2t[:], si12, Bb, AL.mult)
            tt(T1t[:], T1t[:], T2t[:], AL.add)
            tt(DSUM[:], T1t[:, 0:128], T1t[:, 128:256], AL.add)
            stt(DM[:, 0:128], DSUM[:], 2.0, KT[:], AL.mult, AL.add)
            # clamp: this d2 form can round negative near circle contact
            # (the reference's sum-of-squares form cannot); below ~1e-8 it
            # has no relative accuracy anyway.
            ts(DM[:, 0:128], DM[:, 0:128], 1e-8, None,
               AL.max).then_inc(s_d, 1)                    # s_d -> 3k+1
            # fill while ACT computes the rd seed:
            tt(DM[:, 128:256], DM[:, 0:128], SD[:, 0:128], AL.is_lt)
            ts(DM[:, 128:256], DM[:, 128:256], -float(k))
            # ---- rd = rsqrt(d2); g ----
            vector.wait_ge(s_a, 4 * k + 3)
            seedf = SEEDI[:].bitcast(F32)
            cubic_rsqrt(RD[:], seedf, DM[:, 0:128], Q1[:], P1q[:])
            rdrep = _blocks(RD[:], 0, [[0, 2], [1, 128]])
            tt(G[:], Nt[:], rdrep, AL.mult)
            # ---- Adam state ----
            if it["i"] == 0:
                ts(MM[:], MM[:], 0.0)
                ts(VV[:], VV[:], 0.0)
            stt(MM[:], MM[:], float(B1), G[:], AL.mult, AL.add)
            tt(GSQ[:], G[:], G[:], AL.mult)
            stt(VV[:], VV[:], float(B2), GSQ[:], AL.mult, AL.add)
            ts(XT[:], VV[:], it["sigma"], 1e-16, AL.mult,
               AL.add).then_inc(s_d, 1)                    # s_d -> 3k+2
            # ---- z = rsqrt(xt); update ----
            vector.wait_ge(s_a, 4 * k + 4)
            zseedf = ZSEEDI[:].bitcast(F32)
            cubic_rsqrt(ZZ[:], zseedf, XT[:], QZ[:], PZq[:])
            stt(UPD[:], MM[:], it["alpha"], ZZ[:], AL.mult, AL.mult)
            tt(TT[:], TT[:], UPD[:], AL.subtract)
            if last:
                tt(SD[:], SD[:], DM[:], AL.min).then_inc(s_d, 1)
            else:
                # ASIN = t wrapped into [-pi, pi]  (|t| < 2pi always)
                ts(C1t[:], TT[:], PI, -TWO_PI, AL.is_gt, AL.mult)
                ts(C2t[:], TT[:], -PI, TWO_PI, AL.is_lt, AL.mult)
                tt(S_[:], C1t[:], C2t[:], AL.add)
                tt(ASIN[:], TT[:], S_[:], AL.add).then_inc(s_d, 1)  # -> 3k+3
                # deferred, fills the gap while ACT runs Sins of k+1:
                tt(SD[:], SD[:], DM[:], AL.min)

    ctx.close()
    return nc


_BUILD_CACHE = {}


def _get_built(num_iter):
    if num_iter not in _BUILD_CACHE:
        _BUILD_CACHE[num_iter] = build_kernel(num_iter)
    return _BUILD_CACHE[num_iter]


def kernel(c1, r1, z1, c2, r2, z2, num_iter):
    num_iter = int(num_iter)
    c1 = np.asarray(c1, F); r1 = np.asarray(r1, F); z1 = np.asarray(z1, F)
    c2 = np.asarray(c2, F); r2 = np.asarray(r2, F); z2 = np.asarray(z2, F)
    N = c1.shape[0]
    per = N // NCORES
    assert per == P * FD, f"kernel hardcodes {P*FD} pairs/core, got {per}"

    t1, t2, cst, (x1, y1, x2, y2) = _host_precompute(c1, r1, z1, c2, r2, z2)

    def shard_pack(*qs):
        out = []
        for c in range(NCORES):
            sl = slice(c * per, (c + 1) * per)
            out.append(np.concatenate(
                [q[sl].reshape(P, FD) for q in qs], axis=1))
        return out

    A1, Bb1, A2, Bb2 = cst["A1"], cst["Bb1"], cst["A2"], cst["Bb2"]
    U, V, W, X, K = cst["U"], cst["V"], cst["W"], cst["X"], cst["K"]

    ca = shard_pack(U, W, U, V)
    cb = shard_pack(V, X, W, X)
    cd = shard_pack(A1, Bb1, -A2, -Bb2)
    ab = shard_pack(A1, Bb1)
    kt = shard_pack(K)
    tt0 = shard_pack(t1, t2)
    mgk = np.broadcast_to(
        np.array([[float(MAGIC), -0.5, PI / 2, -1.0]], F), (P, 4)).copy()

    in_maps = [
        {"ca": ca[c], "cb": cb[c], "cd": cd[c], "ab": ab[c], "kt": kt[c],
         "tt0": tt0[c], "mgk": mgk}
        for c in range(NCORES)
    ]

    nc = _get_built(num_iter)
    trace = os.environ.get("BASS_KERNEL_TRACE", "0") == "1"
    if trace:
        _install_ntff_hook()
    res = run_bass_kernel_spmd(nc, in_maps, core_ids=list(range(NCORES)),
                               trace=trace)
    if trace and res.exec_time_ns is not None:
        print(f"HW exec time: {res.exec_time_ns} ns")

    t1f = np.empty(N, F); t2f = np.empty(N, F)
    bd = np.empty(N, F); bi = np.empty(N, np.int32)
    co1 = np.empty(N, F); si1 = np.empty(N, F)
    co2 = np.empty(N, F); si2 = np.empty(N, F)
    for c in range(NCORES):
        sl = slice(c * per, (c + 1) * per)
        r = res.results[c]
        t12 = r["t12"]; sdv = r["sd"]; tg = r["tg"]
        t1f[sl] = t12[:, 0:128].reshape(-1)
        t2f[sl] = t12[:, 128:256].reshape(-1)
        bd[sl] = np.sqrt(sdv[:, 0:128].astype(np.float64)).astype(F).reshape(-1)
        bi[sl] = (-sdv[:, 128:256].reshape(-1)).astype(np.int32)
        co2[sl] = tg[:, 0:128].reshape(-1)
        co1[sl] = tg[:, 128:256].reshape(-1)
        si2[sl] = tg[:, 256:384].reshape(-1)
        si1[sl] = tg[:, 384:512].reshape(-1)

    p1 = c1 + r1[:, None] * (co1[:, None] * x1 + si1[:, None] * y1)
    p2 = c2 + r2[:, None] * (co2[:, None] * x2 + si2[:, None] * y2)
    return (bd, bi, t1f, t2f, p1.astype(F), p2.astype(F))


# revision 11
# speedup vs baseline: 1.2421x; 1.0231x over previous
"""Trainium2 Bass kernel for nn_MinDistTwoCircles.

kernel(**inputs) takes FULL unsharded inputs (c1,r1,z1,c2,r2,z2,num_iter)
and returns the FULL output tuple (bd, bi, t1, t2, p1, p2), matching
reference.reference().

Strategy:
  - Host (numpy): circle axes, initial angles (arctan2), per-pair dot
    constants; final p1/p2 reconstruction. O(N) once.
  - Device (8 NeuronCores, data-parallel over the pair axis): the
    2*num_iter Adam iterations. Per core 16384 pairs as [128 x 128] f32
    tiles; quantities packed into column blocks so most vector ops run at
    FD 256-512. Per iteration: comparison-based angle wrap -> ACT Sin
    (only ACT table ever loaded) -> bilinear geometry via block streams ->
    rsqrt(d2) and rsqrt(sigma*V + eps^2) via ACT bitcast magic seed + 2
    Newton steps -> Adam update. All state stays in SBUF.

Raw bass (no Tile): the toolchain here rejects instructions with >1 sem
wait, so the kernel uses explicit single-wait ping-pong semaphores:
per iteration DVE incs (d2, xt, tw) alternate with ACT incs (sins,
rd-seed, z-seed).
"""
import os
import sys
import types
import numpy as np
from contextlib import ExitStack

import concourse.bass as bass
import concourse.mybir as mybir
from concourse.bass_utils import run_bass_kernel_spmd

F = np.float32
P = 128
FD = 128
NCORES = 8
B1, B2, EPS = 0.9, 0.999, 1e-8
MAGIC = 0x5F3759DF
PI = float(np.pi)
TWO_PI = float(2 * np.pi)
F32 = mybir.dt.float32
I32 = mybir.dt.int32
AL = mybir.AluOpType
AF = mybir.ActivationFunctionType


# --------------------------------------------------------------------------
# host-side math
# --------------------------------------------------------------------------
def _normalize(v):
    return v / np.linalg.norm(v, axis=1, keepdims=True).astype(F)


def _axes(normal):
    n = _normalize(normal.astype(F))
    z = np.broadcast_to(np.array([0.0, 0.0, 1.0], F), n.shape).copy()
    boo = (np.sum(n * z, axis=1) < 0.01)[:, None]
    z = np.where(boo, np.array([0.0, 1.0, 0.0], F), z)
    x = _normalize(np.cross(n, z).astype(F))
    y = _normalize(np.cross(n, x).astype(F))
    return n, x, y


def _host_precompute(c1, r1, z1, c2, r2, z2):
    z1n, x1, y1 = _axes(z1)
    z2n, x2, y2 = _axes(z2)
    v = _normalize((c2 - c1).astype(F))
    v1 = _normalize(v - np.sum(v * z1n, axis=1, keepdims=True) * z1n)
    t1 = np.arctan2(np.sum(y1 * v1, axis=1), np.sum(x1 * v1, axis=1)).astype(F)
    v2 = _normalize(-v - np.sum(-v * z2n, axis=1, keepdims=True) * z2n)
    t2 = np.arctan2(np.sum(y2 * v2, axis=1), np.sum(x2 * v2, axis=1)).astype(F)

    u1 = r1[:, None] * x1
    w1 = r1[:, None] * y1
    u2 = r2[:, None] * x2
    w2 = r2[:, None] * y2
    e = (c1 - c2).astype(F)
    dot = lambda a, b: np.sum(a * b, axis=1).astype(F)
    cst = dict(
        A1=dot(e, u1), Bb1=dot(e, w1), A2=dot(e, u2), Bb2=dot(e, w2),
        U=dot(u1, u2), V=dot(u1, w2), W=dot(w1, u2), X=dot(w1, w2),
        K=(dot(e, e) + r1 * r1 + r2 * r2).astype(F),
    )
    return t1, t2, cst, (x1, y1, x2, y2)


# --------------------------------------------------------------------------
# optional NTFF profiling hook
# --------------------------------------------------------------------------
def _install_ntff_hook():
    try:
        if "antenv.axon_hooks" not in sys.modules:
            mod = types.ModuleType("antenv.axon_hooks")
            mod._hook = None
            mod.set_axon_ntff_profile_hook = lambda h: setattr(mod, "_hook", h)
            mod.get_axon_ntff_profile_hook = lambda: mod._hook
            import antenv
            antenv.axon_hooks = mod
            sys.modules["antenv.axon_hooks"] = mod
        from trn_agent_boot.trn_boot import _ntff_profile_via_ctypes
        sys.modules["antenv.axon_hooks"].set_axon_ntff_profile_hook(
            _ntff_profile_via_ctypes("/opt/axon/libaxon_pjrt.so"))
        return True
    except Exception:
        return False


# --------------------------------------------------------------------------
# device kernel
# --------------------------------------------------------------------------
def _blocks(tile_ap, offset_cols, dims):
    """AP keeping the partition dim, free dims replaced by [step,count]
    pairs (element units), starting at column offset_cols."""
    return bass.AP(
        tensor=tile_ap.tensor,
        offset=tile_ap.offset + offset_cols,
        ap=[list(tile_ap.ap[0])] + [list(d) for d in dims],
    )


def build_kernel(num_iter: int):
    # detect_race_conditions=False: the CoreSim race detector has no model of
    # same-engine in-order retirement (verified safe on HW), and this kernel
    # chains dependent ops on one engine constantly.
    nc = bass.Bass("TRN2", debug=False, detect_race_conditions=False)

    ca_d = nc.dram_tensor("ca", [P, 512], F32, kind="ExternalInput")
    cb_d = nc.dram_tensor("cb", [P, 512], F32, kind="ExternalInput")
    cd_d = nc.dram_tensor("cd", [P, 512], F32, kind="ExternalInput")
    ab_d = nc.dram_tensor("ab", [P, 256], F32, kind="ExternalInput")
    kt_d = nc.dram_tensor("kt", [P, 128], F32, kind="ExternalInput")
    tt_d = nc.dram_tensor("tt0", [P, 256], F32, kind="ExternalInput")
    mg_d = nc.dram_tensor("mgk", [P, 4], F32, kind="ExternalInput")

    t12_o = nc.dram_tensor("t12", [P, 256], F32, kind="ExternalOutput")
    sd_o = nc.dram_tensor("sd", [P, 256], F32, kind="ExternalOutput")
    tg_o = nc.dram_tensor("tg", [P, 512], F32, kind="ExternalOutput")

    ctx = ExitStack()
    sb = lambda name, cols, dt=F32: ctx.enter_context(
        nc.sbuf_tensor(name, [P, cols], dt))

    CA = sb("CA", 512); CB = sb("CB", 512); CD = sb("CD", 512)
    BIGX = sb("BIGX", 768)
    KT = sb("KT", 128)
    TT = sb("TTs", 256); ASIN = sb("ASIN", 256)
    MG = sb("MG", 4)
    MM = sb("MM", 256); VV = sb("VV", 256)
    SD = sb("SD", 256)
    G = sb("G", 256); GSQ = sb("GSQ", 256)
    TRIG4 = sb("TRIG4", 512)
    ABS = sb("ABS", 256)
    PPt = sb("PPt", 512); QQt = sb("QQt", 512)
    PROD = sb("PROD", 512); DSUM = sb("DSUM", 128)
    T1t = sb("T1t", 256); T2t = sb("T2t", 256); Nt = sb("Nt", 256)
    DM = sb("DM", 256)
    SEEDI = sb("SEEDI", 128, I32)
    ZSEEDI = sb("ZSEEDI", 256, I32)
    RD = sb("RD", 128); Q1 = sb("Q1", 128); P1q = sb("P1q", 128)
    ZZ = sb("ZZ", 256); QZ = sb("QZ", 256); PZq = sb("PZq", 256)
    XT = sb("XT", 256)
    UPD = sb("UPD", 256)
    C1t = sb("C1t", 256); C2t = sb("C2t", 256)
    S_ = sb("S_", 256)

    dma = ctx.enter_context(nc.semaphore())
    s_d = ctx.enter_context(nc.semaphore())
    s_a = ctx.enter_context(nc.semaphore())
    s_g = ctx.enter_context(nc.semaphore())
    block = ctx.enter_context(nc.Block())

    iters = []
    lr = 0.1
    for phase in range(2):
        lr = lr / 10.0
        for i in range(num_iter):
            st = i + 1
            bc1 = 1 - B1 ** st
            bc2 = 1 - B2 ** st
            iters.append(dict(
                k=phase * num_iter + i, i=i,
                alpha=float(F(lr * (1 - B1) / bc1)),
                sigma=float(F((1 - B2) / bc2)),
            ))
    NIT = len(iters)
    BD2_INIT = float(F(99999.0) * F(99999.0))
    N_LOADS = 8

    @block.sync
    def _(sync):
        sync.dma_start(CA[:], ca_d.ap()[:]).then_inc(dma, 16)
        sync.dma_start(CB[:], cb_d.ap()[:]).then_inc(dma, 16)
        sync.dma_start(CD[:], cd_d.ap()[:]).then_inc(dma, 16)
        sync.dma_start(BIGX[:, 512:768], ab_d.ap()[:]).then_inc(dma, 16)
        sync.dma_start(KT[:], kt_d.ap()[:]).then_inc(dma, 16)
        sync.dma_start(TT[:], tt_d.ap()[:]).then_inc(dma, 16)
        sync.dma_start(ASIN[:], tt_d.ap()[:]).then_inc(dma, 16)
        sync.dma_start(MG[:], mg_d.ap()[:]).then_inc(dma, 16)
        sync.wait_ge(s_d, 3 * NIT)
        sync.dma_start(t12_o.ap()[:], TT[:]).then_inc(dma, 16)
        sync.dma_start(sd_o.ap()[:], SD[:]).then_inc(dma, 16)
        sync.dma_start(tg_o.ap()[:], TRIG4[:]).then_inc(dma, 16)

    @block.gpsimd
    def _(gpsimd):
        gpsimd.memset(MM[:], 0.0)
        gpsimd.memset(VV[:], 0.0)
        gpsimd.memset(SD[:, 128:256], 0.0)
        gpsimd.memset(SD[:, 0:128], BD2_INIT).then_inc(s_g, 1)

    @block.scalar
    def _(scalar):
        mg_magic = MG[:, 0:1]
        mg_mhalf = MG[:, 1:2]
        mg_pih = MG[:, 2:3]
        mg_m1 = MG[:, 3:4]
        for it in iters:
            k = it["k"]
            if k == 0:
                scalar.wait_ge(dma, 16 * N_LOADS)
            else:
                scalar.wait_ge(s_d, 3 * k)
            # sin half: TRIG4[256:512] = [si2|si1]
            twswap = _blocks(ASIN[:], 128, [[-128, 2], [1, 128]])
            nc.scalar.activation(TRIG4[:, 256:512], twswap, AF.Sin)
            # cos half: TRIG4[0:256] = Sin(pi/2 - [|tw2|,|tw1|]) = [co2|co1]
            nc.scalar.activation(ABS[:], ASIN[:], AF.Abs)
            absswap = _blocks(ABS[:], 128, [[-128, 2], [1, 128]])
            nc.scalar.activation(TRIG4[:, 0:256], absswap, AF.Sin,
                                 bias=mg_pih, scale=mg_m1).then_inc(s_a, 2)
            scalar.wait_ge(s_d, 3 * k + 1)
            nc.scalar.activation(SEEDI[:], DM[:, 0:128].bitcast(I32),
                                 AF.Identity, bias=mg_magic,
                                 scale=mg_mhalf).then_inc(s_a, 1)
            scalar.wait_ge(s_d, 3 * k + 2)
            nc.scalar.activation(ZSEEDI[:], XT[:].bitcast(I32),
                                 AF.Identity, bias=mg_magic,
                                 scale=mg_mhalf).then_inc(s_a, 1)

    @block.vector
    def _(vector):
        vector.wait_ge(s_g, 1)

        def tt(out, a, b, op):
            return nc.vector.tensor_tensor(out=out, in0=a, in1=b, op=op)

        def ts(out, a, s1, s2=None, op0=AL.mult, op1=None):
            if op1 is None:
                return nc.vector.tensor_scalar(out=out, in0=a, scalar1=s1,
                                               scalar2=None, op0=op0)
            return nc.vector.tensor_scalar(out=out, in0=a, scalar1=s1,
                                           scalar2=s2, op0=op0, op1=op1)

        def stt(out, a, s, b, op0, op1):
            return nc.vector.scalar_tensor_tensor(out=out, in0=a, scalar=s,
                                                  in1=b, op0=op0, op1=op1)

        def cubic_rsqrt(ynew, y, x, q, p):
            # one 2nd-order Householder step: y*(15/8 - 5/4 q + 3/8 q^2)
            tt(q, y, y, AL.mult)
            tt(q, q, x, AL.mult)
            ts(p, q, 0.375, -1.25, AL.mult, AL.add)
            tt(p, p, q, AL.mult)
            ts(p, p, 1.875, None, AL.add)
            return tt(ynew, y, p, AL.mult)

        for it in iters:
            k = it["k"]
            last = (k == NIT - 1)
            # ---- geometry ----
            vector.wait_ge(s_a, 4 * k + 2)
            si4 = _blocks(TRIG4[:], 256, [[128, 2], [0, 2], [1, 128]])
            tt(QQt[:], si4, CB[:], AL.mult)
            tt(PPt[:], CD[:], QQt[:], AL.subtract)
            co4 = _blocks(TRIG4[:], 0, [[128, 2], [0, 2], [1, 128]])
            tt(QQt[:], co4, CA[:], AL.mult)
            tt(BIGX[:, 0:512], PPt[:], QQt[:], AL.subtract)
            co12 = _blocks(TRIG4[:], 128, [[-128, 2], [1, 128]])
            si12 = _blocks(TRIG4[:], 384, [[-128, 2], [1, 128]])
            b12 = _blocks(BIGX[:], 128, [[256, 2], [1, 128]])
            a12 = _blocks(BIGX[:], 0, [[256, 2], [1, 128]])
            tt(T1t[:], co12, b12, AL.mult)
            tt(T2t[:], si12, a12, AL.mult)
            tt(Nt[:], T1t[:], T2t[:], AL.subtract)
            # d2 = 2*(co1*A1 + si1*B1 + co2*a2' + si2*b2') + K via pair ops
            Aa = _blocks(BIGX[:], 512, [[-256, 2], [1, 128]])
            Bb = _blocks(BIGX[:], 640, [[-256, 2], [1, 128]])
            tt(T1t[:], co12, Aa, AL.mult)
            tt(T2t[:], si12, Bb, AL.mult)
            tt(T1t[:], T1t[:], T2t[:], AL.add)
            tt(DSUM[:], T1t[:, 0:128], T1t[:, 128:256], AL.add)
            stt(DM[:, 0:128], DSUM[:], 2.0, KT[:], AL.mult, AL.add)
            # clamp: this d2 form can round negative near circle contact
            # (the reference's sum-of-squares form cannot); below ~1e-8 it
            # has no relative accuracy anyway.
            ts(DM[:, 0:128], DM[:, 0:128], 1e-8, None,
               AL.max).then_inc(s_d, 1)                    # s_d -> 3k+1
            # fill while ACT computes the rd seed:
            tt(DM[:, 128:256], DM[:, 0:128], SD[:, 0:128], AL.is_lt)
            ts(DM[:, 128:256], DM[:, 128:256], -float(k))
            # ---- rd = rsqrt(d2); g ----
            vector.wait_ge(s_a, 4 * k + 3)
            seedf = SEEDI[:].bitcast(F32)
            cubic_rsqrt(RD[:], seedf, DM[:, 0:128], Q1[:], P1q[:])
            rdrep = _blocks(RD[:], 0, [[0, 2], [1, 128]])
            tt(G[:], Nt[:], rdrep, AL.mult)
            # ---- Adam state ----
            if it["i"] == 0:
                ts(MM[:], MM[:], 0.0)
                ts(VV[:], VV[:], 0.0)
            tt(GSQ[:], G[:], G[:], AL.mult)
            stt(VV[:], VV[:], float(B2), GSQ[:], AL.mult, AL.add)
            ts(XT[:], VV[:], it["sigma"], 1e-16, AL.mult,
               AL.add).then_inc(s_d, 1)                    # s_d -> 3k+2
            # fill the z-seed gap with the M update (independent of XT):
            stt(MM[:], MM[:], float(B1), G[:], AL.mult, AL.add)
            # ---- z = rsqrt(xt); update ----
            vector.wait_ge(s_a, 4 * k + 4)
            zseedf = ZSEEDI[:].bitcast(F32)
            cubic_rsqrt(ZZ[:], zseedf, XT[:], QZ[:], PZq[:])
            stt(UPD[:], MM[:], it["alpha"], ZZ[:], AL.mult, AL.mult)
            tt(TT[:], TT[:], UPD[:], AL.subtract)
            if last:
                tt(SD[:], SD[:], DM[:], AL.min).then_inc(s_d, 1)
            else:
                # ASIN = t wrapped into [-pi, pi]  (|t| < 2pi always)
                ts(C1t[:], TT[:], PI, -TWO_PI, AL.is_gt, AL.mult)
                ts(C2t[:], TT[:], -PI, TWO_PI, AL.is_lt, AL.mult)
                tt(S_[:], C1t[:], C2t[:], AL.add)
                tt(ASIN[:], TT[:], S_[:], AL.add).then_inc(s_d, 1)  # -> 3k+3
                # deferred, fills the gap while ACT runs Sins of k+1:
                tt(SD[:], SD[:], DM[:], AL.min)

    ctx.close()
    return nc


_BUILD_CACHE = {}


def _get_built(num_iter):
    if num_iter not in _BUILD_CACHE:
        _BUILD_CACHE[num_iter] = build_kernel(num_iter)
    return _BUILD_CACHE[num_iter]


def kernel(c1, r1, z1, c2, r2, z2, num_iter):
    num_iter = int(num_iter)
    c1 = np.asarray(c1, F); r1 = np.asarray(r1, F); z1 = np.asarray(z1, F)
    c2 = np.asarray(c2, F); r2 = np.asarray(r2, F); z2 = np.asarray(z2, F)
    N = c1.shape[0]
    per = N // NCORES
    assert per == P * FD, f"kernel hardcodes {P*FD} pairs/core, got {per}"

    t1, t2, cst, (x1, y1, x2, y2) = _host_precompute(c1, r1, z1, c2, r2, z2)

    def shard_pack(*qs):
        out = []
        for c in range(NCORES):
            sl = slice(c * per, (c + 1) * per)
            out.append(np.concatenate(
                [q[sl].reshape(P, FD) for q in qs], axis=1))
        return out

    A1, Bb1, A2, Bb2 = cst["A1"], cst["Bb1"], cst["A2"], cst["Bb2"]
    U, V, W, X, K = cst["U"], cst["V"], cst["W"], cst["X"], cst["K"]

    ca = shard_pack(U, W, U, V)
    cb = shard_pack(V, X, W, X)
    cd = shard_pack(A1, Bb1, -A2, -Bb2)
    ab = shard_pack(A1, Bb1)
    kt = shard_pack(K)
    tt0 = shard_pack(t1, t2)
    mgk = np.broadcast_to(
        np.array([[float(MAGIC), -0.5, PI / 2, -1.0]], F), (P, 4)).copy()

    in_maps = [
        {"ca": ca[c], "cb": cb[c], "cd": cd[c], "ab": ab[c], "kt": kt[c],
         "tt0": tt0[c], "mgk": mgk}
        for c in range(NCORES)
    ]

    nc = _get_built(num_iter)
    trace = os.environ.get("BASS_KERNEL_TRACE", "0") == "1"
    if trace:
        _install_ntff_hook()
    res = run_bass_kernel_spmd(nc, in_maps, core_ids=list(range(NCORES)),
                               trace=trace)
    if trace and res.exec_time_ns is not None:
        print(f"HW exec time: {res.exec_time_ns} ns")

    t1f = np.empty(N, F); t2f = np.empty(N, F)
    bd = np.empty(N, F); bi = np.empty(N, np.int32)
    co1 = np.empty(N, F); si1 = np.empty(N, F)
    co2 = np.empty(N, F); si2 = np.empty(N, F)
    for c in range(NCORES):
        sl = slice(c * per, (c + 1) * per)
        r = res.results[c]
        t12 = r["t12"]; sdv = r["sd"]; tg = r["tg"]
        t1f[sl] = t12[:, 0:128].reshape(-1)
        t2f[sl] = t12[:, 128:256].reshape(-1)
        bd[sl] = np.sqrt(sdv[:, 0:128].astype(np.float64)).astype(F).reshape(-1)
        bi[sl] = (-sdv[:, 128:256].reshape(-1)).astype(np.int32)
        co2[sl] = tg[:, 0:128].reshape(-1)
        co1[sl] = tg[:, 128:256].reshape(-1)
        si2[sl] = tg[:, 256:384].reshape(-1)
        si1[sl] = tg[:, 384:512].reshape(-1)

    p1 = c1 + r1[:, None] * (co1[:, None] * x1 + si1[:, None] * y1)
    p2 = c2 + r2[:, None] * (co2[:, None] * x2 + si2[:, None] * y2)
    return (bd, bi, t1f, t2f, p1.astype(F), p2.astype(F))


# revision 12
# speedup vs baseline: 1.2440x; 1.0015x over previous
"""Trainium2 Bass kernel for nn_MinDistTwoCircles.

kernel(**inputs) takes FULL unsharded inputs (c1,r1,z1,c2,r2,z2,num_iter)
and returns the FULL output tuple (bd, bi, t1, t2, p1, p2), matching
reference.reference().

Strategy:
  - Host (numpy): circle axes, initial angles (arctan2), per-pair dot
    constants; final p1/p2 reconstruction. O(N) once.
  - Device (8 NeuronCores, data-parallel over the pair axis): the
    2*num_iter Adam iterations. Per core 16384 pairs as [128 x 128] f32
    tiles; quantities packed into column blocks so most vector ops run at
    FD 256-512. Per iteration: comparison-based angle wrap -> ACT Sin
    (only ACT table ever loaded) -> bilinear geometry via block streams ->
    rsqrt(d2) and rsqrt(sigma*V + eps^2) via ACT bitcast magic seed + 2
    Newton steps -> Adam update. All state stays in SBUF.

Raw bass (no Tile): the toolchain here rejects instructions with >1 sem
wait, so the kernel uses explicit single-wait ping-pong semaphores:
per iteration DVE incs (d2, xt, tw) alternate with ACT incs (sins,
rd-seed, z-seed).
"""
import os
import sys
import types
import numpy as np
from contextlib import ExitStack

import concourse.bass as bass
import concourse.mybir as mybir
from concourse.bass_utils import run_bass_kernel_spmd

F = np.float32
P = 128
FD = 128
NCORES = 8
B1, B2, EPS = 0.9, 0.999, 1e-8
MAGIC = 0x5F3759DF
PI = float(np.pi)
TWO_PI = float(2 * np.pi)
F32 = mybir.dt.float32
I32 = mybir.dt.int32
AL = mybir.AluOpType
AF = mybir.ActivationFunctionType


# --------------------------------------------------------------------------
# host-side math
# --------------------------------------------------------------------------
def _normalize(v):
    return v / np.linalg.norm(v, axis=1, keepdims=True).astype(F)


def _axes(normal):
    n = _normalize(normal.astype(F))
    z = np.broadcast_to(np.array([0.0, 0.0, 1.0], F), n.shape).copy()
    boo = (np.sum(n * z, axis=1) < 0.01)[:, None]
    z = np.where(boo, np.array([0.0, 1.0, 0.0], F), z)
    x = _normalize(np.cross(n, z).astype(F))
    y = _normalize(np.cross(n, x).astype(F))
    return n, x, y


def _host_precompute(c1, r1, z1, c2, r2, z2):
    z1n, x1, y1 = _axes(z1)
    z2n, x2, y2 = _axes(z2)
    v = _normalize((c2 - c1).astype(F))
    v1 = _normalize(v - np.sum(v * z1n, axis=1, keepdims=True) * z1n)
    t1 = np.arctan2(np.sum(y1 * v1, axis=1), np.sum(x1 * v1, axis=1)).astype(F)
    v2 = _normalize(-v - np.sum(-v * z2n, axis=1, keepdims=True) * z2n)
    t2 = np.arctan2(np.sum(y2 * v2, axis=1), np.sum(x2 * v2, axis=1)).astype(F)

    u1 = r1[:, None] * x1
    w1 = r1[:, None] * y1
    u2 = r2[:, None] * x2
    w2 = r2[:, None] * y2
    e = (c1 - c2).astype(F)
    dot = lambda a, b: np.sum(a * b, axis=1).astype(F)
    cst = dict(
        A1=dot(e, u1), Bb1=dot(e, w1), A2=dot(e, u2), Bb2=dot(e, w2),
        U=dot(u1, u2), V=dot(u1, w2), W=dot(w1, u2), X=dot(w1, w2),
        K=(dot(e, e) + r1 * r1 + r2 * r2).astype(F),
    )
    return t1, t2, cst, (x1, y1, x2, y2)


# --------------------------------------------------------------------------
# optional NTFF profiling hook
# --------------------------------------------------------------------------
def _install_ntff_hook():
    try:
        if "antenv.axon_hooks" not in sys.modules:
            mod = types.ModuleType("antenv.axon_hooks")
            mod._hook = None
            mod.set_axon_ntff_profile_hook = lambda h: setattr(mod, "_hook", h)
            mod.get_axon_ntff_profile_hook = lambda: mod._hook
            import antenv
            antenv.axon_hooks = mod
            sys.modules["antenv.axon_hooks"] = mod
        from trn_agent_boot.trn_boot import _ntff_profile_via_ctypes
        sys.modules["antenv.axon_hooks"].set_axon_ntff_profile_hook(
            _ntff_profile_via_ctypes("/opt/axon/libaxon_pjrt.so"))
        return True
    except Exception:
        return False


# --------------------------------------------------------------------------
# device kernel
# --------------------------------------------------------------------------
def _blocks(tile_ap, offset_cols, dims):
    """AP keeping the partition dim, free dims replaced by [step,count]
    pairs (element units), starting at column offset_cols."""
    return bass.AP(
        tensor=tile_ap.tensor,
        offset=tile_ap.offset + offset_cols,
        ap=[list(tile_ap.ap[0])] + [list(d) for d in dims],
    )


def build_kernel(num_iter: int):
    # detect_race_conditions=False: the CoreSim race detector has no model of
    # same-engine in-order retirement (verified safe on HW), and this kernel
    # chains dependent ops on one engine constantly.
    nc = bass.Bass("TRN2", debug=False, detect_race_conditions=False)

    ca_d = nc.dram_tensor("ca", [P, 512], F32, kind="ExternalInput")
    cb_d = nc.dram_tensor("cb", [P, 512], F32, kind="ExternalInput")
    cd_d = nc.dram_tensor("cd", [P, 512], F32, kind="ExternalInput")
    ab_d = nc.dram_tensor("ab", [P, 256], F32, kind="ExternalInput")
    kt_d = nc.dram_tensor("kt", [P, 128], F32, kind="ExternalInput")
    tt_d = nc.dram_tensor("tt0", [P, 256], F32, kind="ExternalInput")
    mg_d = nc.dram_tensor("mgk", [P, 4], F32, kind="ExternalInput")

    t12_o = nc.dram_tensor("t12", [P, 256], F32, kind="ExternalOutput")
    sd_o = nc.dram_tensor("sd", [P, 256], F32, kind="ExternalOutput")
    tg_o = nc.dram_tensor("tg", [P, 512], F32, kind="ExternalOutput")

    ctx = ExitStack()
    sb = lambda name, cols, dt=F32: ctx.enter_context(
        nc.sbuf_tensor(name, [P, cols], dt))

    CA = sb("CA", 512); CB = sb("CB", 512); CD = sb("CD", 512)
    BIGX = sb("BIGX", 768)
    KT = sb("KT", 128)
    TT = sb("TTs", 256); ASIN = sb("ASIN", 256)
    MG = sb("MG", 4)
    MM = sb("MM", 256); VV = sb("VV", 256)
    SD = sb("SD", 256)
    G = sb("G", 256); GSQ = sb("GSQ", 256)
    TRIG4 = sb("TRIG4", 512)
    ABS = sb("ABS", 256)
    PPt = sb("PPt", 512); QQt = sb("QQt", 512)
    PROD = sb("PROD", 512); DSUM = sb("DSUM", 128)
    T1t = sb("T1t", 256); T2t = sb("T2t", 256); Nt = sb("Nt", 256)
    DM = sb("DM", 256)
    SEEDI = sb("SEEDI", 128, I32); SEEDF = sb("SEEDF", 128)
    ZSEEDI = sb("ZSEEDI", 256, I32); ZSEEDF = sb("ZSEEDF", 256)
    RD = sb("RD", 128); Q1 = sb("Q1", 128); P1q = sb("P1q", 128)
    ZZ = sb("ZZ", 256); QZ = sb("QZ", 256); PZq = sb("PZq", 256)
    XT = sb("XT", 256)
    UPD = sb("UPD", 256)
    C1t = sb("C1t", 256); C2t = sb("C2t", 256)
    S_ = sb("S_", 256)

    dma = ctx.enter_context(nc.semaphore())
    s_d = ctx.enter_context(nc.semaphore())
    s_a = ctx.enter_context(nc.semaphore())
    s_g = ctx.enter_context(nc.semaphore())
    block = ctx.enter_context(nc.Block())

    iters = []
    lr = 0.1
    for phase in range(2):
        lr = lr / 10.0
        for i in range(num_iter):
            st = i + 1
            bc1 = 1 - B1 ** st
            bc2 = 1 - B2 ** st
            iters.append(dict(
                k=phase * num_iter + i, i=i,
                alpha=float(F(lr * (1 - B1) / bc1)),
                sigma=float(F((1 - B2) / bc2)),
            ))
    NIT = len(iters)
    BD2_INIT = float(F(99999.0) * F(99999.0))
    N_LOADS = 8
    MAGICF = float(MAGIC)

    @block.sync
    def _(sync):
        sync.dma_start(CA[:], ca_d.ap()[:]).then_inc(dma, 16)
        sync.dma_start(CB[:], cb_d.ap()[:]).then_inc(dma, 16)
        sync.dma_start(CD[:], cd_d.ap()[:]).then_inc(dma, 16)
        sync.dma_start(BIGX[:, 512:768], ab_d.ap()[:]).then_inc(dma, 16)
        sync.dma_start(KT[:], kt_d.ap()[:]).then_inc(dma, 16)
        sync.dma_start(TT[:], tt_d.ap()[:]).then_inc(dma, 16)
        sync.dma_start(ASIN[:], tt_d.ap()[:]).then_inc(dma, 16)
        sync.dma_start(MG[:], mg_d.ap()[:]).then_inc(dma, 16)
        sync.wait_ge(s_d, NIT)
        sync.dma_start(t12_o.ap()[:], TT[:]).then_inc(dma, 16)
        sync.dma_start(sd_o.ap()[:], SD[:]).then_inc(dma, 16)
        sync.dma_start(tg_o.ap()[:], TRIG4[:]).then_inc(dma, 16)

    @block.gpsimd
    def _(gpsimd):
        gpsimd.memset(MM[:], 0.0)
        gpsimd.memset(VV[:], 0.0)
        gpsimd.memset(SD[:, 128:256], 0.0)
        gpsimd.memset(SD[:, 0:128], BD2_INIT).then_inc(s_g, 1)

    @block.scalar
    def _(scalar):
        mg_magic = MG[:, 0:1]
        mg_mhalf = MG[:, 1:2]
        mg_pih = MG[:, 2:3]
        mg_m1 = MG[:, 3:4]
        for it in iters:
            k = it["k"]
            if k == 0:
                scalar.wait_ge(dma, 16 * N_LOADS)
            else:
                scalar.wait_ge(s_d, k)
            # sin half: TRIG4[256:512] = [si2|si1]
            twswap = _blocks(ASIN[:], 128, [[-128, 2], [1, 128]])
            nc.scalar.activation(TRIG4[:, 256:512], twswap, AF.Sin)
            # cos half: TRIG4[0:256] = Sin(pi/2 - [|tw2|,|tw1|]) = [co2|co1]
            nc.scalar.activation(ABS[:], ASIN[:], AF.Abs)
            absswap = _blocks(ABS[:], 128, [[-128, 2], [1, 128]])
            nc.scalar.activation(TRIG4[:, 0:256], absswap, AF.Sin,
                                 bias=mg_pih, scale=mg_m1).then_inc(s_a, 1)

    @block.vector
    def _(vector):
        vector.wait_ge(s_g, 1)

        def tt(out, a, b, op):
            return nc.vector.tensor_tensor(out=out, in0=a, in1=b, op=op)

        def ts(out, a, s1, s2=None, op0=AL.mult, op1=None):
            if op1 is None:
                return nc.vector.tensor_scalar(out=out, in0=a, scalar1=s1,
                                               scalar2=None, op0=op0)
            return nc.vector.tensor_scalar(out=out, in0=a, scalar1=s1,
                                           scalar2=s2, op0=op0, op1=op1)

        def stt(out, a, s, b, op0, op1):
            return nc.vector.scalar_tensor_tensor(out=out, in0=a, scalar=s,
                                                  in1=b, op0=op0, op1=op1)

        def cubic_rsqrt(ynew, y, x, q, p):
            # one 2nd-order Householder step: y*(15/8 - 5/4 q + 3/8 q^2)
            tt(q, y, y, AL.mult)
            tt(q, q, x, AL.mult)
            ts(p, q, 0.375, -1.25, AL.mult, AL.add)
            tt(p, p, q, AL.mult)
            ts(p, p, 1.875, None, AL.add)
            return tt(ynew, y, p, AL.mult)

        for it in iters:
            k = it["k"]
            last = (k == NIT - 1)
            # ---- geometry ----
            vector.wait_ge(s_a, k + 1)
            si4 = _blocks(TRIG4[:], 256, [[128, 2], [0, 2], [1, 128]])
            tt(QQt[:], si4, CB[:], AL.mult)
            tt(PPt[:], CD[:], QQt[:], AL.subtract)
            co4 = _blocks(TRIG4[:], 0, [[128, 2], [0, 2], [1, 128]])
            tt(QQt[:], co4, CA[:], AL.mult)
            tt(BIGX[:, 0:512], PPt[:], QQt[:], AL.subtract)
            co12 = _blocks(TRIG4[:], 128, [[-128, 2], [1, 128]])
            si12 = _blocks(TRIG4[:], 384, [[-128, 2], [1, 128]])
            b12 = _blocks(BIGX[:], 128, [[256, 2], [1, 128]])
            a12 = _blocks(BIGX[:], 0, [[256, 2], [1, 128]])
            tt(T1t[:], co12, b12, AL.mult)
            tt(T2t[:], si12, a12, AL.mult)
            tt(Nt[:], T1t[:], T2t[:], AL.subtract)
            # d2 = 2*(co1*A1 + si1*B1 + co2*a2' + si2*b2') + K via pair ops
            Aa = _blocks(BIGX[:], 512, [[-256, 2], [1, 128]])
            Bb = _blocks(BIGX[:], 640, [[-256, 2], [1, 128]])
            tt(T1t[:], co12, Aa, AL.mult)
            tt(T2t[:], si12, Bb, AL.mult)
            tt(T1t[:], T1t[:], T2t[:], AL.add)
            tt(DSUM[:], T1t[:, 0:128], T1t[:, 128:256], AL.add)
            stt(DM[:, 0:128], DSUM[:], 2.0, KT[:], AL.mult, AL.add)
            # clamp: this d2 form can round negative near circle contact
            # (the reference's sum-of-squares form cannot); below ~1e-8 it
            # has no relative accuracy anyway.
            ts(DM[:, 0:128], DM[:, 0:128], 1e-8, None, AL.max)
            tt(DM[:, 128:256], DM[:, 0:128], SD[:, 0:128], AL.is_lt)
            ts(DM[:, 128:256], DM[:, 128:256], -float(k))
            # ---- rd = rsqrt(d2); g ----  (magic seed via convert-copies)
            nc.vector.tensor_copy(SEEDF[:], DM[:, 0:128].bitcast(I32))
            ts(SEEDF[:], SEEDF[:], -0.5, MAGICF, AL.mult, AL.add)
            nc.vector.tensor_copy(SEEDI[:], SEEDF[:])
            seedf = SEEDI[:].bitcast(F32)
            cubic_rsqrt(RD[:], seedf, DM[:, 0:128], Q1[:], P1q[:])
            rdrep = _blocks(RD[:], 0, [[0, 2], [1, 128]])
            tt(G[:], Nt[:], rdrep, AL.mult)
            # ---- Adam state ----
            if it["i"] == 0:
                ts(MM[:], MM[:], 0.0)
                ts(VV[:], VV[:], 0.0)
            tt(GSQ[:], G[:], G[:], AL.mult)
            stt(VV[:], VV[:], float(B2), GSQ[:], AL.mult, AL.add)
            ts(XT[:], VV[:], it["sigma"], 1e-16, AL.mult, AL.add)
            stt(MM[:], MM[:], float(B1), G[:], AL.mult, AL.add)
            # ---- z = rsqrt(xt); update ----  (seed on DVE)
            nc.vector.tensor_copy(ZSEEDF[:], XT[:].bitcast(I32))
            ts(ZSEEDF[:], ZSEEDF[:], -0.5, MAGICF, AL.mult, AL.add)
            nc.vector.tensor_copy(ZSEEDI[:], ZSEEDF[:])
            zseedf = ZSEEDI[:].bitcast(F32)
            cubic_rsqrt(ZZ[:], zseedf, XT[:], QZ[:], PZq[:])
            stt(UPD[:], MM[:], it["alpha"], ZZ[:], AL.mult, AL.mult)
            tt(TT[:], TT[:], UPD[:], AL.subtract)
            if last:
                tt(SD[:], SD[:], DM[:], AL.min).then_inc(s_d, 1)
            elif True:
                # ASIN = t wrapped into [-pi, pi]  (|t| < 2pi always)
                ts(C1t[:], TT[:], PI, -TWO_PI, AL.is_gt, AL.mult)
                ts(C2t[:], TT[:], -PI, TWO_PI, AL.is_lt, AL.mult)
                tt(S_[:], C1t[:], C2t[:], AL.add)
                tt(ASIN[:], TT[:], S_[:], AL.add).then_inc(s_d, 1)  # -> k+1
                # deferred, fills the gap while ACT runs Sins of k+1:
                tt(SD[:], SD[:], DM[:], AL.min)

    ctx.close()
    return nc


_BUILD_CACHE = {}


def _get_built(num_iter):
    if num_iter not in _BUILD_CACHE:
        _BUILD_CACHE[num_iter] = build_kernel(num_iter)
    return _BUILD_CACHE[num_iter]


def kernel(c1, r1, z1, c2, r2, z2, num_iter):
    num_iter = int(num_iter)
    c1 = np.asarray(c1, F); r1 = np.asarray(r1, F); z1 = np.asarray(z1, F)
    c2 = np.asarray(c2, F); r2 = np.asarray(r2, F); z2 = np.asarray(z2, F)
    N = c1.shape[0]
    per = N // NCORES
    assert per == P * FD, f"kernel hardcodes {P*FD} pairs/core, got {per}"

    t1, t2, cst, (x1, y1, x2, y2) = _host_precompute(c1, r1, z1, c2, r2, z2)

    def shard_pack(*qs):
        out = []
        for c in range(NCORES):
            sl = slice(c * per, (c + 1) * per)
            out.append(np.concatenate(
                [q[sl].reshape(P, FD) for q in qs], axis=1))
        return out

    A1, Bb1, A2, Bb2 = cst["A1"], cst["Bb1"], cst["A2"], cst["Bb2"]
    U, V, W, X, K = cst["U"], cst["V"], cst["W"], cst["X"], cst["K"]

    ca = shard_pack(U, W, U, V)
    cb = shard_pack(V, X, W, X)
    cd = shard_pack(A1, Bb1, -A2, -Bb2)
    ab = shard_pack(A1, Bb1)
    kt = shard_pack(K)
    tt0 = shard_pack(t1, t2)
    mgk = np.broadcast_to(
        np.array([[float(MAGIC), -0.5, PI / 2, -1.0]], F), (P, 4)).copy()

    in_maps = [
        {"ca": ca[c], "cb": cb[c], "cd": cd[c], "ab": ab[c], "kt": kt[c],
         "tt0": tt0[c], "mgk": mgk}
        for c in range(NCORES)
    ]

    nc = _get_built(num_iter)
    trace = os.environ.get("BASS_KERNEL_TRACE", "0") == "1"
    if trace:
        _install_ntff_hook()
    res = run_bass_kernel_spmd(nc, in_maps, core_ids=list(range(NCORES)),
                               trace=trace)
    if trace and res.exec_time_ns is not None:
        print(f"HW exec time: {res.exec_time_ns} ns")

    t1f = np.empty(N, F); t2f = np.empty(N, F)
    bd = np.empty(N, F); bi = np.empty(N, np.int32)
    co1 = np.empty(N, F); si1 = np.empty(N, F)
    co2 = np.empty(N, F); si2 = np.empty(N, F)
    for c in range(NCORES):
        sl = slice(c * per, (c + 1) * per)
        r = res.results[c]
        t12 = r["t12"]; sdv = r["sd"]; tg = r["tg"]
        t1f[sl] = t12[:, 0:128].reshape(-1)
        t2f[sl] = t12[:, 128:256].reshape(-1)
        bd[sl] = np.sqrt(sdv[:, 0:128].astype(np.float64)).astype(F).reshape(-1)
        bi[sl] = (-sdv[:, 128:256].reshape(-1)).astype(np.int32)
        co2[sl] = tg[:, 0:128].reshape(-1)
        co1[sl] = tg[:, 128:256].reshape(-1)
        si2[sl] = tg[:, 256:384].reshape(-1)
        si1[sl] = tg[:, 384:512].reshape(-1)

    p1 = c1 + r1[:, None] * (co1[:, None] * x1 + si1[:, None] * y1)
    p2 = c2 + r2[:, None] * (co2[:, None] * x2 + si2[:, None] * y2)
    return (bd, bi, t1f, t2f, p1.astype(F), p2.astype(F))


# revision 13
# speedup vs baseline: 1.2743x; 1.0243x over previous
"""Trainium2 Bass kernel for nn_MinDistTwoCircles.

kernel(**inputs) takes FULL unsharded inputs (c1,r1,z1,c2,r2,z2,num_iter)
and returns the FULL output tuple (bd, bi, t1, t2, p1, p2), matching
reference.reference().

Strategy:
  - Host (numpy): circle axes, initial angles (arctan2), per-pair dot
    constants; final p1/p2 reconstruction. O(N) once.
  - Device (8 NeuronCores, data-parallel over the pair axis): the
    2*num_iter Adam iterations. Per core 16384 pairs as [128 x 128] f32
    tiles; quantities packed into column blocks so most vector ops run at
    FD 256-512. Per iteration: comparison-based angle wrap -> ACT Sin
    (only ACT table ever loaded) -> bilinear geometry via block streams ->
    rsqrt(d2) and rsqrt(sigma*V + eps^2) via ACT bitcast magic seed + 2
    Newton steps -> Adam update. All state stays in SBUF.

Raw bass (no Tile): the toolchain here rejects instructions with >1 sem
wait, so the kernel uses explicit single-wait ping-pong semaphores:
per iteration DVE incs (d2, xt, tw) alternate with ACT incs (sins,
rd-seed, z-seed).
"""
import os
import sys
import types
import numpy as np
from contextlib import ExitStack

import concourse.bass as bass
import concourse.mybir as mybir
from concourse.bass_utils import run_bass_kernel_spmd

F = np.float32
P = 128
FD = 128
NCORES = 8
B1, B2, EPS = 0.9, 0.999, 1e-8
MAGIC = 0x5F3759DF
PI = float(np.pi)
TWO_PI = float(2 * np.pi)
F32 = mybir.dt.float32
I32 = mybir.dt.int32
AL = mybir.AluOpType
AF = mybir.ActivationFunctionType


# --------------------------------------------------------------------------
# host-side math
# --------------------------------------------------------------------------
def _normalize(v):
    return v / np.linalg.norm(v, axis=1, keepdims=True).astype(F)


def _axes(normal):
    n = _normalize(normal.astype(F))
    z = np.broadcast_to(np.array([0.0, 0.0, 1.0], F), n.shape).copy()
    boo = (np.sum(n * z, axis=1) < 0.01)[:, None]
    z = np.where(boo, np.array([0.0, 1.0, 0.0], F), z)
    x = _normalize(np.cross(n, z).astype(F))
    y = _normalize(np.cross(n, x).astype(F))
    return n, x, y


def _host_precompute(c1, r1, z1, c2, r2, z2):
    z1n, x1, y1 = _axes(z1)
    z2n, x2, y2 = _axes(z2)
    v = _normalize((c2 - c1).astype(F))
    v1 = _normalize(v - np.sum(v * z1n, axis=1, keepdims=True) * z1n)
    t1 = np.arctan2(np.sum(y1 * v1, axis=1), np.sum(x1 * v1, axis=1)).astype(F)
    v2 = _normalize(-v - np.sum(-v * z2n, axis=1, keepdims=True) * z2n)
    t2 = np.arctan2(np.sum(y2 * v2, axis=1), np.sum(x2 * v2, axis=1)).astype(F)

    u1 = r1[:, None] * x1
    w1 = r1[:, None] * y1
    u2 = r2[:, None] * x2
    w2 = r2[:, None] * y2
    e = (c1 - c2).astype(F)
    dot = lambda a, b: np.sum(a * b, axis=1).astype(F)
    cst = dict(
        A1=dot(e, u1), Bb1=dot(e, w1), A2=dot(e, u2), Bb2=dot(e, w2),
        U=dot(u1, u2), V=dot(u1, w2), W=dot(w1, u2), X=dot(w1, w2),
        K=(dot(e, e) + r1 * r1 + r2 * r2).astype(F),
    )
    return t1, t2, cst, (x1, y1, x2, y2)


# --------------------------------------------------------------------------
# optional NTFF profiling hook
# --------------------------------------------------------------------------
def _install_ntff_hook():
    try:
        if "antenv.axon_hooks" not in sys.modules:
            mod = types.ModuleType("antenv.axon_hooks")
            mod._hook = None
            mod.set_axon_ntff_profile_hook = lambda h: setattr(mod, "_hook", h)
            mod.get_axon_ntff_profile_hook = lambda: mod._hook
            import antenv
            antenv.axon_hooks = mod
            sys.modules["antenv.axon_hooks"] = mod
        from trn_agent_boot.trn_boot import _ntff_profile_via_ctypes
        sys.modules["antenv.axon_hooks"].set_axon_ntff_profile_hook(
            _ntff_profile_via_ctypes("/opt/axon/libaxon_pjrt.so"))
        return True
    except Exception:
        return False


# --------------------------------------------------------------------------
# device kernel
# --------------------------------------------------------------------------
def _blocks(tile_ap, offset_cols, dims):
    """AP keeping the partition dim, free dims replaced by [step,count]
    pairs (element units), starting at column offset_cols."""
    return bass.AP(
        tensor=tile_ap.tensor,
        offset=tile_ap.offset + offset_cols,
        ap=[list(tile_ap.ap[0])] + [list(d) for d in dims],
    )


def build_kernel(num_iter: int):
    # detect_race_conditions=False: the CoreSim race detector has no model of
    # same-engine in-order retirement (verified safe on HW), and this kernel
    # chains dependent ops on one engine constantly.
    nc = bass.Bass("TRN2", debug=False, detect_race_conditions=False)

    ca_d = nc.dram_tensor("ca", [P, 512], F32, kind="ExternalInput")
    cb_d = nc.dram_tensor("cb", [P, 512], F32, kind="ExternalInput")
    cd_d = nc.dram_tensor("cd", [P, 512], F32, kind="ExternalInput")
    ab_d = nc.dram_tensor("ab", [P, 256], F32, kind="ExternalInput")
    kt_d = nc.dram_tensor("kt", [P, 128], F32, kind="ExternalInput")
    tt_d = nc.dram_tensor("tt0", [P, 256], F32, kind="ExternalInput")
    mg_d = nc.dram_tensor("mgk", [P, 4], F32, kind="ExternalInput")

    t12_o = nc.dram_tensor("t12", [P, 256], F32, kind="ExternalOutput")
    sd_o = nc.dram_tensor("sd", [P, 256], F32, kind="ExternalOutput")
    tg_o = nc.dram_tensor("tg", [P, 512], F32, kind="ExternalOutput")

    ctx = ExitStack()
    sb = lambda name, cols, dt=F32: ctx.enter_context(
        nc.sbuf_tensor(name, [P, cols], dt))

    CA = sb("CA", 512); CB = sb("CB", 512); CD = sb("CD", 512)
    BIGX = sb("BIGX", 768)
    KT = sb("KT", 128)
    TT = sb("TTs", 256); ASIN = sb("ASIN", 256)
    MG = sb("MG", 4)
    MM = sb("MM", 256); VV = sb("VV", 256)
    SD = sb("SD", 256)
    G = sb("G", 256); GSQ = sb("GSQ", 256)
    TRIG4 = sb("TRIG4", 512)
    ABS = sb("ABS", 256)
    PPt = sb("PPt", 512); QQt = sb("QQt", 512)
    PROD = sb("PROD", 512); DSUM = sb("DSUM", 128)
    T1t = sb("T1t", 256); T2t = sb("T2t", 256); Nt = sb("Nt", 256)
    DM = sb("DM", 256)
    SEEDI = sb("SEEDI", 128, I32); SEEDF = sb("SEEDF", 128)
    ZSEEDI = sb("ZSEEDI", 256, I32); ZSEEDF = sb("ZSEEDF", 256)
    RD = sb("RD", 128); Q1 = sb("Q1", 128); P1q = sb("P1q", 128)
    ZZ = sb("ZZ", 256); QZ = sb("QZ", 256); PZq = sb("PZq", 256)
    XT = sb("XT", 256)
    UPD = sb("UPD", 256)
    C1t = sb("C1t", 256); C2t = sb("C2t", 256)
    S_ = sb("S_", 256)

    dma = ctx.enter_context(nc.semaphore())
    s_d = ctx.enter_context(nc.semaphore())
    s_a = ctx.enter_context(nc.semaphore())
    s_g = ctx.enter_context(nc.semaphore())
    block = ctx.enter_context(nc.Block())

    iters = []
    lr = 0.1
    for phase in range(2):
        lr = lr / 10.0
        for i in range(num_iter):
            st = i + 1
            bc1 = 1 - B1 ** st
            bc2 = 1 - B2 ** st
            iters.append(dict(
                k=phase * num_iter + i, i=i,
                alpha=float(F(lr * (1 - B1) / bc1)),
                sigma=float(F((1 - B2) / bc2)),
            ))
    NIT = len(iters)
    BD2_INIT = float(F(99999.0) * F(99999.0))
    N_LOADS = 8
    MAGICF = float(MAGIC)

    @block.sync
    def _(sync):
        sync.dma_start(CA[:], ca_d.ap()[:]).then_inc(dma, 16)
        sync.dma_start(CB[:], cb_d.ap()[:]).then_inc(dma, 16)
        sync.dma_start(CD[:], cd_d.ap()[:]).then_inc(dma, 16)
        sync.dma_start(BIGX[:, 512:768], ab_d.ap()[:]).then_inc(dma, 16)
        sync.dma_start(KT[:], kt_d.ap()[:]).then_inc(dma, 16)
        sync.dma_start(TT[:], tt_d.ap()[:]).then_inc(dma, 16)
        sync.dma_start(ASIN[:], tt_d.ap()[:]).then_inc(dma, 16)
        sync.dma_start(MG[:], mg_d.ap()[:]).then_inc(dma, 16)
        sync.wait_ge(s_d, NIT)
        sync.dma_start(t12_o.ap()[:], TT[:]).then_inc(dma, 16)
        sync.dma_start(sd_o.ap()[:], SD[:]).then_inc(dma, 16)
        sync.dma_start(tg_o.ap()[:], TRIG4[:]).then_inc(dma, 16)

    @block.gpsimd
    def _(gpsimd):
        gpsimd.memset(MM[:], 0.0)
        gpsimd.memset(VV[:], 0.0)
        gpsimd.memset(SD[:, 128:256], 0.0)
        gpsimd.memset(SD[:, 0:128], BD2_INIT).then_inc(s_g, 1)

    @block.scalar
    def _(scalar):
        mg_magic = MG[:, 0:1]
        mg_mhalf = MG[:, 1:2]
        mg_pih = MG[:, 2:3]
        mg_m1 = MG[:, 3:4]
        for it in iters:
            k = it["k"]
            if k == 0:
                scalar.wait_ge(dma, 16 * N_LOADS)
            else:
                scalar.wait_ge(s_d, k)
            # sin half: TRIG4[256:512] = [si2|si1]
            twswap = _blocks(ASIN[:], 128, [[-128, 2], [1, 128]])
            nc.scalar.activation(TRIG4[:, 256:512], twswap, AF.Sin)
            # cos half: TRIG4[0:256] = Sin(pi/2 - [|tw2|,|tw1|]) = [co2|co1]
            nc.scalar.activation(ABS[:], ASIN[:], AF.Abs)
            absswap = _blocks(ABS[:], 128, [[-128, 2], [1, 128]])
            nc.scalar.activation(TRIG4[:, 0:256], absswap, AF.Sin,
                                 bias=mg_pih, scale=mg_m1).then_inc(s_a, 1)

    @block.vector
    def _(vector):
        vector.wait_ge(s_g, 1)

        def tt(out, a, b, op):
            return nc.vector.tensor_tensor(out=out, in0=a, in1=b, op=op)

        def ts(out, a, s1, s2=None, op0=AL.mult, op1=None):
            if op1 is None:
                return nc.vector.tensor_scalar(out=out, in0=a, scalar1=s1,
                                               scalar2=None, op0=op0)
            return nc.vector.tensor_scalar(out=out, in0=a, scalar1=s1,
                                           scalar2=s2, op0=op0, op1=op1)

        def stt(out, a, s, b, op0, op1):
            return nc.vector.scalar_tensor_tensor(out=out, in0=a, scalar=s,
                                                  in1=b, op0=op0, op1=op1)

        def cubic_rsqrt(ynew, y, x, q, p):
            # one 2nd-order Householder step: y*(15/8 - 5/4 q + 3/8 q^2)
            tt(q, y, y, AL.mult)
            tt(q, q, x, AL.mult)
            ts(p, q, 0.375, -1.25, AL.mult, AL.add)
            tt(p, p, q, AL.mult)
            ts(p, p, 1.875, None, AL.add)
            return tt(ynew, y, p, AL.mult)

        for it in iters:
            k = it["k"]
            last = (k == NIT - 1)
            # ---- geometry ----
            vector.wait_ge(s_a, k + 1)
            si4 = _blocks(TRIG4[:], 256, [[128, 2], [0, 2], [1, 128]])
            tt(QQt[:], si4, CB[:], AL.mult)
            tt(PPt[:], CD[:], QQt[:], AL.subtract)
            co4 = _blocks(TRIG4[:], 0, [[128, 2], [0, 2], [1, 128]])
            tt(QQt[:], co4, CA[:], AL.mult)
            tt(BIGX[:, 0:512], PPt[:], QQt[:], AL.subtract)
            co12 = _blocks(TRIG4[:], 128, [[-128, 2], [1, 128]])
            si12 = _blocks(TRIG4[:], 384, [[-128, 2], [1, 128]])
            b12 = _blocks(BIGX[:], 128, [[256, 2], [1, 128]])
            a12 = _blocks(BIGX[:], 0, [[256, 2], [1, 128]])
            tt(T1t[:], co12, b12, AL.mult)
            tt(T2t[:], si12, a12, AL.mult)
            tt(Nt[:], T1t[:], T2t[:], AL.subtract)
            # d2 = 2*(co1*A1 + si1*B1 + co2*a2' + si2*b2') + K via pair ops
            Aa = _blocks(BIGX[:], 512, [[-256, 2], [1, 128]])
            Bb = _blocks(BIGX[:], 640, [[-256, 2], [1, 128]])
            tt(T1t[:], co12, Aa, AL.mult)
            tt(T2t[:], si12, Bb, AL.mult)
            tt(T1t[:], T1t[:], T2t[:], AL.add)
            tt(DSUM[:], T1t[:, 0:128], T1t[:, 128:256], AL.add)
            stt(DM[:, 0:128], DSUM[:], 2.0, KT[:], AL.mult, AL.add)
            # clamp: this d2 form can round negative near circle contact
            # (the reference's sum-of-squares form cannot); below ~1e-8 it
            # has no relative accuracy anyway.
            ts(DM[:, 0:128], DM[:, 0:128], 1e-8, None, AL.max)
            tt(DM[:, 128:256], DM[:, 0:128], SD[:, 0:128], AL.is_lt)
            ts(DM[:, 128:256], DM[:, 128:256], -float(k))
            # ---- rd = rsqrt(d2); g ----  (magic seed via convert-copies)
            nc.vector.tensor_copy(SEEDF[:], DM[:, 0:128].bitcast(I32))
            ts(SEEDI[:], SEEDF[:], -0.5, MAGICF, AL.mult, AL.add)
            seedf = SEEDI[:].bitcast(F32)
            cubic_rsqrt(RD[:], seedf, DM[:, 0:128], Q1[:], P1q[:])
            rdrep = _blocks(RD[:], 0, [[0, 2], [1, 128]])
            tt(G[:], Nt[:], rdrep, AL.mult)
            # ---- Adam state ----
            if it["i"] == 0:
                ts(MM[:], MM[:], 0.0)
                ts(VV[:], VV[:], 0.0)
            tt(GSQ[:], G[:], G[:], AL.mult)
            stt(VV[:], VV[:], float(B2), GSQ[:], AL.mult, AL.add)
            ts(XT[:], VV[:], it["sigma"], 1e-16, AL.mult, AL.add)
            stt(MM[:], MM[:], float(B1), G[:], AL.mult, AL.add)
            # ---- z = rsqrt(xt); update ----  (seed on DVE)
            nc.vector.tensor_copy(ZSEEDF[:], XT[:].bitcast(I32))
            ts(ZSEEDI[:], ZSEEDF[:], -0.5, MAGICF, AL.mult, AL.add)
            zseedf = ZSEEDI[:].bitcast(F32)
            cubic_rsqrt(ZZ[:], zseedf, XT[:], QZ[:], PZq[:])
            stt(UPD[:], MM[:], it["alpha"], ZZ[:], AL.mult, AL.mult)
            tt(TT[:], TT[:], UPD[:], AL.subtract)
            if last:
                tt(SD[:], SD[:], DM[:], AL.min).then_inc(s_d, 1)
            elif True:
                # ASIN = t wrapped into [-pi, pi]  (|t| < 2pi always)
                ts(C1t[:], TT[:], PI, -TWO_PI, AL.is_gt, AL.mult)
                ts(C2t[:], TT[:], -PI, TWO_PI, AL.is_lt, AL.mult)
                tt(S_[:], C1t[:], C2t[:], AL.add)
                tt(ASIN[:], TT[:], S_[:], AL.add).then_inc(s_d, 1)  # -> k+1
                # deferred, fills the gap while ACT runs Sins of k+1:
                tt(SD[:], SD[:], DM[:], AL.min)

    ctx.close()
    return nc


_BUILD_CACHE = {}


def _get_built(num_iter):
    if num_iter not in _BUILD_CACHE:
        _BUILD_CACHE[num_iter] = build_kernel(num_iter)
    return _BUILD_CACHE[num_iter]


def kernel(c1, r1, z1, c2, r2, z2, num_iter):
    num_iter = int(num_iter)
    c1 = np.asarray(c1, F); r1 = np.asarray(r1, F); z1 = np.asarray(z1, F)
    c2 = np.asarray(c2, F); r2 = np.asarray(r2, F); z2 = np.asarray(z2, F)
    N = c1.shape[0]
    per = N // NCORES
    assert per == P * FD, f"kernel hardcodes {P*FD} pairs/core, got {per}"

    t1, t2, cst, (x1, y1, x2, y2) = _host_precompute(c1, r1, z1, c2, r2, z2)

    def shard_pack(*qs):
        out = []
        for c in range(NCORES):
            sl = slice(c * per, (c + 1) * per)
            out.append(np.concatenate(
                [q[sl].reshape(P, FD) for q in qs], axis=1))
        return out

    A1, Bb1, A2, Bb2 = cst["A1"], cst["Bb1"], cst["A2"], cst["Bb2"]
    U, V, W, X, K = cst["U"], cst["V"], cst["W"], cst["X"], cst["K"]

    ca = shard_pack(U, W, U, V)
    cb = shard_pack(V, X, W, X)
    cd = shard_pack(A1, Bb1, -A2, -Bb2)
    ab = shard_pack(A1, Bb1)
    kt = shard_pack(K)
    tt0 = shard_pack(t1, t2)
    mgk = np.broadcast_to(
        np.array([[float(MAGIC), -0.5, PI / 2, -1.0]], F), (P, 4)).copy()

    in_maps = [
        {"ca": ca[c], "cb": cb[c], "cd": cd[c], "ab": ab[c], "kt": kt[c],
         "tt0": tt0[c], "mgk": mgk}
        for c in range(NCORES)
    ]

    nc = _get_built(num_iter)
    trace = os.environ.get("BASS_KERNEL_TRACE", "0") == "1"
    if trace:
        _install_ntff_hook()
    res = run_bass_kernel_spmd(nc, in_maps, core_ids=list(range(NCORES)),
                               trace=trace)
    if trace and res.exec_time_ns is not None:
        print(f"HW exec time: {res.exec_time_ns} ns")

    t1f = np.empty(N, F); t2f = np.empty(N, F)
    bd = np.empty(N, F); bi = np.empty(N, np.int32)
    co1 = np.empty(N, F); si1 = np.empty(N, F)
    co2 = np.empty(N, F); si2 = np.empty(N, F)
    for c in range(NCORES):
        sl = slice(c * per, (c + 1) * per)
        r = res.results[c]
        t12 = r["t12"]; sdv = r["sd"]; tg = r["tg"]
        t1f[sl] = t12[:, 0:128].reshape(-1)
        t2f[sl] = t12[:, 128:256].reshape(-1)
        bd[sl] = np.sqrt(sdv[:, 0:128].astype(np.float64)).astype(F).reshape(-1)
        bi[sl] = (-sdv[:, 128:256].reshape(-1)).astype(np.int32)
        co2[sl] = tg[:, 0:128].reshape(-1)
        co1[sl] = tg[:, 128:256].reshape(-1)
        si2[sl] = tg[:, 256:384].reshape(-1)
        si1[sl] = tg[:, 384:512].reshape(-1)

    p1 = c1 + r1[:, None] * (co1[:, None] * x1 + si1[:, None] * y1)
    p2 = c2 + r2[:, None] * (co2[:, None] * x2 + si2[:, None] * y2)
    return (bd, bi, t1f, t2f, p1.astype(F), p2.astype(F))
